# revision 19
# baseline (speedup 1.0000x reference)
"""BiLSTM-CRF forward loss on 8 Trainium2 NeuronCores.

Strategy: pure data-parallel over batch (8 sequences per core, no
cross-core communication).  Per core:
  1. embedding gather (indirect DMA) + PE-transpose -> x.T feature-major
  2. input-projection GEMMs (fp32r, full rate at N=512) -> u = x @ Wih.T + b
  3. two BiLSTM layers: forward+backward scans interleaved on the same
     core; recurrent matmuls in bf16 with weights stationary
     (feature-major h, no transposes in the loop)
  4. emission GEMM (fp32r) -> emissions feature-major [17, T*8]
  5. CRF partition function via a blocked exp-domain scan (8 parallel
     chunk products, log-domain combine) + numerator via masked one-hot
     contractions.  Each core returns 5 scalars; the host combines them.
"""

import sys

for _p in ("/opt/trn_rl_repo", "/root/.axon_site/_ro/trn_rl_repo"):
    if _p not in sys.path:
        sys.path.insert(0, _p)

import numpy as np

import concourse.bass as bass
import concourse.bacc as bacc
import concourse.mybir as mybir
import concourse.tile as tile
from concourse.bass import ds, ts
from concourse.bass_utils import run_bass_kernel_spmd
from concourse.masks import make_identity

F32 = mybir.dt.float32
F32R = mybir.dt.float32r
BF16 = mybir.dt.bfloat16
I32 = mybir.dt.int32
U8 = mybir.dt.uint8
AF = mybir.ActivationFunctionType
OP = mybir.AluOpType
AX = mybir.AxisListType

B = 8            # sequences per core
E = 512          # embedding dim
H = 512          # hidden per direction
G = 2048         # 4*H gate rows
L = 17           # number of tags
CREB = 2.83      # CRF exp-domain rebase constant (log-domain growth/step)
CC = 8           # CRF chunk count

_CACHE = {}


def _build(T, V):
    NT = T * B                 # tokens per core, time-major cols (t*B + b)
    NG = NT // 128             # gather tiles of 128 tokens
    NCH = T // 8               # scan chunks of 8 timesteps
    W = min(512, NT)           # GEMM window (moving free dim)
    NW = NT // W
    CL = T // CC               # CRF chunk length
    NC2 = NT // 128            # 128-col chunks for the trans-count matmul

    nc = bacc.Bacc(None, target_bir_lowering=False, debug=False)

    # ---------------- kernel I/O ----------------
    emb_in = nc.dram_tensor("emb", [V, E], F32, kind="ExternalInput")
    gidx_in = nc.dram_tensor("gidx", [128, NG], I32, kind="ExternalInput")
    wih1f_in = nc.dram_tensor("wih1f", [E, G], F32, kind="ExternalInput")
    wih1b_in = nc.dram_tensor("wih1b", [E, G], F32, kind="ExternalInput")
    whh1f_in = nc.dram_tensor("whh1f", [H, G], F32, kind="ExternalInput")
    whh1b_in = nc.dram_tensor("whh1b", [H, G], F32, kind="ExternalInput")
    wih2f_in = nc.dram_tensor("wih2f", [2 * H, G], F32, kind="ExternalInput")
    wih2b_in = nc.dram_tensor("wih2b", [2 * H, G], F32, kind="ExternalInput")
    whh2f_in = nc.dram_tensor("whh2f", [H, G], F32, kind="ExternalInput")
    whh2b_in = nc.dram_tensor("whh2b", [H, G], F32, kind="ExternalInput")
    b1f_in = nc.dram_tensor("b1f", [128, 16], F32, kind="ExternalInput")
    b1b_in = nc.dram_tensor("b1b", [128, 16], F32, kind="ExternalInput")
    b2f_in = nc.dram_tensor("b2f", [128, 16], F32, kind="ExternalInput")
    b2b_in = nc.dram_tensor("b2b", [128, 16], F32, kind="ExternalInput")
    wout_in = nc.dram_tensor("wout", [2 * H, L], F32, kind="ExternalInput")
    bout_in = nc.dram_tensor("bout", [L, 1], F32, kind="ExternalInput")
    trans_in = nc.dram_tensor("trans", [L, L], F32, kind="ExternalInput")
    start_in = nc.dram_tensor("start", [L, 1], F32, kind="ExternalInput")
    end_in = nc.dram_tensor("end", [L, 1], F32, kind="ExternalInput")
    wmask_in = nc.dram_tensor("wmask", [L, NT], F32, kind="ExternalInput")
    ohprev_in = nc.dram_tensor("ohprev", [NT, L], F32, kind="ExternalInput")
    ohnext_in = nc.dram_tensor("ohnext", [NT, L], F32, kind="ExternalInput")
    oh0_in = nc.dram_tensor("oh0", [L, B], F32, kind="ExternalInput")
    ohlast_in = nc.dram_tensor("ohlast", [L, B], F32, kind="ExternalInput")
    lenoff_in = nc.dram_tensor("lenoff", [1, B], F32, kind="ExternalInput")
    moff_in = nc.dram_tensor("moff", [L, NT], U8, kind="ExternalInput")

    out_d = nc.dram_tensor("out", [1, 8], F32, kind="ExternalOutput")

    # ---------------- internal DRAM ----------------
    u1f_d = nc.dram_tensor("u1f", [16, 128, NT], BF16)
    u1b_d = nc.dram_tensor("u1b", [16, 128, NT], BF16)
    u2f_d = nc.dram_tensor("u2f", [16, 128, NT], BF16)
    u2b_d = nc.dram_tensor("u2b", [16, 128, NT], BF16)
    h1_d = nc.dram_tensor("h1", [8, 128, NT], BF16)  # rows 0-3 fwd, 4-7 bwd
    h2_d = nc.dram_tensor("h2", [8, 128, NT], BF16)

    with tile.TileContext(nc) as tc:
        _emit(tc, locals(), T=T, V=V, NT=NT, NG=NG, NCH=NCH, W=W, NW=NW,
              CL=CL, NC2=NC2)
    nc.compile()
    return nc


def _emit(tc, d, *, T, V, NT, NG, NCH, W, NW, CL, NC2):
    import os
    PH = int(os.environ.get("BASS_PHASES", "6"))
    nc = tc.nc

    # persistent pool for things that live across phases
    with tc.tile_pool(name="persist", bufs=1) as perst, \
         tc.tile_pool(name="ps_persist", bufs=1, space="PSUM") as psper:

        ident = perst.tile([128, 128], F32)
        make_identity(nc, ident[:])

        # =========== phase 0: gather + transpose -> xT (f32r) ===========
        with tc.tile_pool(name="xtp", bufs=1) as xtp:
            xT = xtp.tile([128, 4, NT], BF16, name="xT")
            with tc.tile_pool(name="p0", bufs=3) as p0, \
                 tc.tile_pool(name="ps0", bufs=4, space="PSUM") as ps0:
                gidx_sb = p0.tile([128, NG], I32, name="gidx")
                nc.sync.dma_start(gidx_sb[:], d["gidx_in"][:])
                for j in range(NG):
                    gx = p0.tile([128, E], F32, name="gx")
                    nc.gpsimd.indirect_dma_start(
                        out=gx[:], out_offset=None, in_=d["emb_in"][:],
                        in_offset=bass.IndirectOffsetOnAxis(
                            ap=gidx_sb[:, j:j + 1], axis=0))
                    for k in range(4):
                        pst = ps0.tile([128, 128], F32, name="ptr")
                        nc.tensor.transpose(pst[:],
                                            gx[:, 128 * k:128 * (k + 1)],
                                            ident[:])
                        nc.vector.tensor_copy(
                            xT[:, k, 128 * j:128 * (j + 1)], pst[:])

            # =========== phase 1: layer-1 input GEMMs ===========
            if PH >= 1:
                _ugemm(tc, d["wih1f_in"], d["b1f_in"], d["u1f_d"], 4, NT, W,
                       NW, rhs_sbuf=xT)
                _ugemm(tc, d["wih1b_in"], d["b1b_in"], d["u1b_d"], 4, NT, W,
                       NW, rhs_sbuf=xT)

        # =========== phase 2: layer-1 scans ===========
        if PH >= 2:
            _scan(tc, d["u1f_d"], d["u1b_d"], d["whh1f_in"], d["whh1b_in"],
                  d["h1_d"], NT, NCH)

        # =========== phase 3: layer-2 input GEMMs ===========
        if PH >= 3:
            _ugemm(tc, d["wih2f_in"], d["b2f_in"], d["u2f_d"], 8, NT, W, NW,
                   rhs_dram=d["h1_d"])
            _ugemm(tc, d["wih2b_in"], d["b2b_in"], d["u2b_d"], 8, NT, W, NW,
                   rhs_dram=d["h1_d"])

        # =========== phase 4: layer-2 scans ===========
        if PH >= 4:
            _scan(tc, d["u2f_d"], d["u2b_d"], d["whh2f_in"], d["whh2b_in"],
                  d["h2_d"], NT, NCH)

        # =========== phase 5: emissions ===========
        if PH < 5:
            with tc.tile_pool(name="stub", bufs=1) as stub:
                zo = stub.tile([1, 8], F32, name="zo")
                nc.vector.memset(zo[:], 0.0)
                nc.sync.dma_start(d["out_d"][:], zo[:])
            return
        em = perst.tile([L, NT], F32, name="em")
        with tc.tile_pool(name="pe", bufs=3) as pe, \
             tc.tile_pool(name="pse", bufs=4, space="PSUM") as pse:
            wo_st = pe.tile([128, 8, L], F32, name="wo_st")
            nc.sync.dma_start(
                wo_st[:], d["wout_in"][:].rearrange("(k p) l -> p k l", p=128))
            wo = pe.tile([128, 8, L], BF16, name="wo")
            nc.vector.tensor_copy(wo[:], wo_st[:])
            bout_sb = pe.tile([L, 1], F32, name="bout")
            nc.sync.dma_start(bout_sb[:], d["bout_in"][:])
            for w in range(NW):
                rh = pe.tile([128, 8, W], BF16, name="rh")
                nc.sync.dma_start(
                    rh[:],
                    d["h2_d"][:].rearrange("k p n -> p k n")[:, :,
                                                            ts(w, W)])
                pst = pse.tile([L, W], F32, name="pem")
                for k in range(8):
                    nc.tensor.matmul(pst[:], wo[:, k, :], rh[:, k, :],
                                     start=(k == 0), stop=(k == 7))
                nc.vector.tensor_add(
                    em[:, ts(w, W)], pst[:],
                    bout_sb[:, :].to_broadcast([L, W]))

        # =========== phase 6: CRF ===========
        if PH >= 6:
            _crf(tc, d, em, perst, psper, T, NT, CL, NC2)
        else:
            with tc.tile_pool(name="stub", bufs=1) as stub:
                zo = stub.tile([1, 8], F32, name="zo")
                nc.vector.memset(zo[:], 0.0)
                nc.sync.dma_start(d["out_d"][:], zo[:])


def _ugemm(tc, w_dram, b_dram, u_dram, KC, NT, W, NW, rhs_sbuf=None,
           rhs_dram=None):
    """u.T[m-tile] = sum_k Wih.T[k,mtile].T @ rhs[k]  (+ bias), fp32r."""
    nc = tc.nc
    with tc.tile_pool(name="ug", bufs=2) as ug, \
         tc.tile_pool(name="ug_w", bufs=1) as ugw, \
         tc.tile_pool(name="ug_ps", bufs=4, space="PSUM") as ups:
        wt = ugw.tile([128, KC, 2048], BF16, name="wt")
        for k in range(KC):
            st = ug.tile([128, 2048], F32, name="wstage")
            nc.sync.dma_start(st[:], w_dram[ds(128 * k, 128), :])
            nc.vector.tensor_copy(wt[:, k, :], st[:])
        bias = ugw.tile([128, 16], F32, name="bias")
        nc.sync.dma_start(bias[:], b_dram[:])
        for w in range(NW):
            if rhs_sbuf is not None:
                rh = rhs_sbuf[:, :, ts(w, W)]
            else:
                rh_t = ug.tile([128, KC, W], BF16, name="rh_t")
                nc.sync.dma_start(
                    rh_t[:],
                    rhs_dram[:].rearrange("k p n -> p k n")[:, :, ts(w, W)])
                rh = rh_t[:]
            for m in range(16):
                pst = ups.tile([128, W], F32, name="pu")
                for k in range(KC):
                    nc.tensor.matmul(pst[:], wt[:, k, 128 * m:128 * (m + 1)],
                                     rh[:, k, :],
                                     start=(k == 0), stop=(k == KC - 1))
                usb = ug.tile([128, W], BF16, name="usb")
                nc.vector.tensor_add(
                    usb[:], pst[:],
                    bias[:, m:m + 1].to_broadcast([128, W]))
                nc.sync.dma_start(u_dram[m, :, ts(w, W)], usb[:])


def _scan(tc, uf_dram, ub_dram, whf_dram, whb_dram, hT_dram, NT, NCH):
    """Interleaved fwd+bwd LSTM scans, bf16 recurrent matmuls."""
    nc = tc.nc
    with tc.tile_pool(name="sc_w", bufs=1) as scw, \
         tc.tile_pool(name="sc_st", bufs=1) as scs, \
         tc.tile_pool(name="sc", bufs=3) as sc, \
         tc.tile_pool(name="sc_ps", bufs=4, space="PSUM") as sps:
        wbf = []
        with tc.tile_pool(name="sc_wst", bufs=1) as scst:
            for wi, w_dram in enumerate((whf_dram, whb_dram)):
                st = scst.tile([128, 4, 2048], F32, name=f"whstage{wi}",
                               tag="whstage")
                nc.sync.dma_start(
                    st[:], w_dram[:].rearrange("(k p) m -> p k m", p=128))
                wb = scw.tile([128, 4, 2048], BF16, name=f"whbf{wi}",
                              tag=f"whbf{wi}")
                nc.vector.tensor_copy(wb[:], st[:])
                wbf.append(wb)

        hst = [scs.tile([128, 4, B], BF16, name=f"h_{i}") for i in range(2)]
        cst = [scs.tile([128, 4, B], F32, name=f"c_{i}") for i in range(2)]
        for t_ in hst + cst:
            nc.vector.memset(t_[:], 0.0)

        uf_ap = uf_dram[:].rearrange("m p n -> p m n")
        ub_ap = ub_dram[:].rearrange("m p n -> p m n")
        hT_f = hT_dram[ds(0, 4)].rearrange("k p n -> p k n")
        hT_b = hT_dram[ds(4, 4)].rearrange("k p n -> p k n")

        with tc.For_i(0, NCH, hint_engines=(mybir.EngineType.PE,),
                      name="scan") as ci:
            u_sb = []
            for di, u_ap in enumerate((uf_ap, ub_ap)):
                ut = sc.tile([128, 16, 64], BF16, name=f"u_{di}")
                if di == 0:
                    nc.sync.dma_start(ut[:], u_ap[:, :, ts(ci, 64)])
                else:
                    nc.sync.dma_start(
                        ut[:], u_ap[:, :, ds(ci * (-64) + (NT - 64), 64)])
                u_sb.append(ut)
            hout = [sc.tile([128, 4, 64], BF16, name=f"ho_{di}")
                    for di in range(2)]
            for j in range(8):
                for di in range(2):
                    h_bf, c_sb = hst[di], cst[di]
                    col = 8 * j if di == 0 else 8 * (7 - j)
                    pst = sps.tile([128, 128], F32, name=f"pg_{di}")
                    for m in range(16):
                        for k in range(4):
                            nc.tensor.matmul(
                                pst[:, 8 * m:8 * (m + 1)],
                                wbf[di][:, k, 128 * m:128 * (m + 1)],
                                h_bf[:, k, :],
                                start=(k == 0), stop=(k == 3))
                    g = sc.tile([128, 128], F32, name=f"g_{di}")
                    nc.vector.tensor_add(
                        g[:].rearrange("p (m b) -> p m b", m=16),
                        pst[:].rearrange("p (m b) -> p m b", m=16),
                        u_sb[di][:, :, ds(col, 8)])
                    sif = sc.tile([128, 64], F32, name=f"sif_{di}")
                    nc.scalar.activation(sif[:], g[:, 0:64], AF.Sigmoid)
                    tg = sc.tile([128, 32], F32, name=f"tg_{di}")
                    nc.scalar.activation(tg[:], g[:, 64:96], AF.Tanh)
                    so = sc.tile([128, 32], F32, name=f"so_{di}")
                    nc.scalar.activation(so[:], g[:, 96:128], AF.Sigmoid)
                    t1 = sc.tile([128, 32], F32, name=f"t1_{di}")
                    nc.vector.tensor_mul(
                        t1[:], sif[:, 32:64],
                        c_sb[:].rearrange("p k b -> p (k b)"))
                    t2 = sc.tile([128, 32], F32, name=f"t2_{di}")
                    nc.vector.tensor_mul(t2[:], sif[:, 0:32], tg[:])
                    nc.vector.tensor_add(
                        c_sb[:].rearrange("p k b -> p (k b)"), t1[:], t2[:])
                    tcn = sc.tile([128, 32], F32, name=f"tc_{di}")
                    nc.scalar.activation(
                        tcn[:], c_sb[:].rearrange("p k b -> p (k b)"),
                        AF.Tanh)
                    nc.vector.tensor_mul(
                        hout[di][:, :, ds(col, 8)],
                        so[:].rearrange("p (k b) -> p k b", k=4),
                        tcn[:].rearrange("p (k b) -> p k b", k=4))
                    nc.vector.tensor_mul(
                        h_bf[:].rearrange("p k b -> p (k b)"),
                        so[:], tcn[:])
            nc.sync.dma_start(hT_f[:, :, ts(ci, 64)], hout[0][:])
            nc.sync.dma_start(hT_b[:, :, ds(ci * (-64) + (NT - 64), 64)],
                              hout[1][:])


def _crf(tc, d, em, perst, psper, T, NT, CL, NC2):
    nc = tc.nc
    L_ = L
    with tc.tile_pool(name="cr", bufs=2) as cr, \
         tc.tile_pool(name="cr_st", bufs=1) as crs, \
         tc.tile_pool(name="cr_ps", bufs=2, space="PSUM") as cps, \
         tc.tile_pool(name="cr_ps1", bufs=1, space="PSUM") as cps1:
        # constants
        trans_sb = crs.tile([L_, L_], F32, name="trans")
        nc.sync.dma_start(trans_sb[:], d["trans_in"][:])
        P_sb = crs.tile([L_, L_], F32, name="P")
        nc.scalar.activation(P_sb[:], trans_sb[:], AF.Exp)
        start_sb = crs.tile([L_, 1], F32, name="start")
        nc.sync.dma_start(start_sb[:], d["start_in"][:])
        end_sb = crs.tile([L_, 1], F32, name="end")
        nc.sync.dma_start(end_sb[:], d["end_in"][:])
        id17 = crs.tile([L_, L_], F32, name="id17")
        make_identity(nc, id17[:])
        one17 = crs.tile([L_, 1], F32, name="one17")
        nc.vector.memset(one17[:], 1.0)
        onerow = crs.tile([1, L_], F32, name="onerow")
        nc.vector.memset(onerow[:], 1.0)
        moff_sb = crs.tile([L_, NT], U8, name="moff")
        nc.sync.dma_start(moff_sb[:], d["moff_in"][:])
        lenoff_sb = crs.tile([1, B], F32, name="lenoff")
        nc.sync.dma_start(lenoff_sb[:], d["lenoff_in"][:])
        out_sb = crs.tile([1, 8], F32, name="outv")
        nc.vector.memset(out_sb[:], 0.0)

        # Emask = exp(em - CREB), 1.0 where step inactive
        crebt = crs.tile([L_, 1], F32, name="crebt")
        nc.vector.memset(crebt[:], -CREB)
        emask = perst.tile([L_, NT], F32, name="emask")
        nc.scalar.activation(emask[:], em[:], AF.Exp, bias=crebt[:, :])
        nc.vector.copy_predicated(emask[:], moff_sb[:],
                                  one17[:, :].to_broadcast([L_, NT]))

        # ---- blocked exp-domain chunk scan ----
        # state At[k, (c, b, i)] = (prod_{tau} P diag(E))^T per chunk c
        NFREE = CC * B * L_
        A = [crs.tile([L_, CC, B, L_], F32, name=f"A{i}") for i in range(2)]
        nc.vector.tensor_copy(
            A[0][:],
            id17[:, None, None, :].to_broadcast([L_, CC, B, L_]))
        NGRP = 4
        CG = CC // NGRP          # chunks per matmul group
        GW = CG * B * L_         # 272
        for tau in range(CL):
            src, dst = A[tau % 2], A[(tau + 1) % 2]
            emsl = emask[:].rearrange("p (c s b) -> p c s b", c=CC,
                                      s=CL)[:, :, tau, :]
            mosl = moff_sb[:].rearrange("p (c s b) -> p c s b", c=CC,
                                        s=CL)[:, :, tau, :]
            for gp in range(NGRP):
                cs = slice(CG * gp, CG * (gp + 1))
                pA = cps.tile([L_, GW], F32, name="pA")
                nc.tensor.matmul(
                    pA[:], P_sb[:],
                    src[:].rearrange("p c b i -> p (c b i)")[:, ts(gp, GW)],
                    start=True, stop=True)
                nc.vector.tensor_mul(
                    dst[:, cs, :, :],
                    pA[:].rearrange("p (c b i) -> p c b i", c=CG, b=B),
                    emsl[:, cs, :, None].to_broadcast([L_, CG, B, L_]))
            nc.vector.copy_predicated(
                dst[:],
                mosl[:, :, :, None].to_broadcast([L_, CC, B, L_]),
                src[:])
        Afin = A[CL % 2]
        logA = perst.tile([L_, CC, B, L_], F32, name="logA")
        nc.scalar.activation(logA[:], Afin[:], AF.Ln)
        nc.vector.tensor_scalar_max(logA[:], logA[:], -1e30)

        # ---- log-domain combine ----
        s_fm = crs.tile([L_, B], F32, name="s_fm")   # s[j, b]
        nc.vector.tensor_add(s_fm[:], em[:, 0:B],
                             start_sb[:, :].to_broadcast([L_, B]))
        s_rep = crs.tile([L_, B, L_], F32, name="s_rep")

        def replicate(src_fm):
            # s_rep[k, (b, i)] = src_fm[i, b] for all k
            pT = cps1.tile([B, L_], F32, name="pT")
            nc.tensor.transpose(pT[:], src_fm[:], id17[:])
            sT = cr.tile([B, L_], F32, name="sT")
            nc.vector.tensor_copy(sT[:], pT[:])
            srow = cr.tile([1, B * L_], F32, name="srow")
            nc.sync.dma_start(srow[:], sT[:])
            pR = cps1.tile([L_, B * L_], F32, name="pR")
            nc.tensor.matmul(pR[:], onerow[:], srow[:], start=True, stop=True)
            nc.vector.tensor_copy(
                s_rep[:], pR[:].rearrange("p (b i) -> p b i", b=B))

        replicate(s_fm)
        for c in range(CC):
            tmp = cr.tile([L_, B, L_], F32, name="ctmp")
            nc.vector.tensor_add(tmp[:], logA[:, c, :, :], s_rep[:])
            etmp = cr.tile([L_, B, L_], F32, name="cetmp")
            nc.scalar.activation(etmp[:], tmp[:], AF.Exp)
            sex = cr.tile([L_, B], F32, name="sex")
            nc.vector.tensor_reduce(sex[:], etmp[:], AX.X, OP.add)
            nc.scalar.activation(s_fm[:], sex[:], AF.Ln)
            if c < CC - 1:
                replicate(s_fm)

        # logZ_b = LSE_k(s[k,b] + end[k]) + lenoff[b]; out slot0 = sum_b
        send = cr.tile([L_, B], F32, name="send")
        nc.vector.tensor_add(send[:], s_fm[:],
                             end_sb[:, :].to_broadcast([L_, B]))
        eend = cr.tile([L_, B], F32, name="eend")
        nc.scalar.activation(eend[:], send[:], AF.Exp)
        pz = cps1.tile([1, B], F32, name="pz")
        nc.tensor.matmul(pz[:], one17[:], eend[:], start=True, stop=True)
        lz = cr.tile([1, B], F32, name="lz")
        nc.scalar.activation(lz[:], pz[:], AF.Ln)
        nc.vector.tensor_add(lz[:], lz[:], lenoff_sb[:])
        nc.vector.tensor_reduce(out_sb[:, 0:1], lz[:], AX.X, OP.add)

        # ---- numerator parts ----
        def dot_to_slot(vec_lp, slot):
            # vec_lp: [L, 1] -> sum over partitions into out_sb[0, slot]
            pd = cps1.tile([1, 1], F32, name="pd")
            nc.tensor.matmul(pd[:], one17[:], vec_lp[:], start=True,
                             stop=True)
            nc.vector.tensor_copy(out_sb[:, slot:slot + 1], pd[:])

        # e_tag part
        wm_sb = cr.tile([L_, NT], F32, name="wm")
        nc.sync.dma_start(wm_sb[:], d["wmask_in"][:])
        prod = cr.tile([L_, NT], F32, name="prod")
        nc.vector.tensor_mul(prod[:], em[:], wm_sb[:])
        r1 = cr.tile([L_, 1], F32, name="r1")
        nc.vector.tensor_reduce(r1[:], prod[:], AX.X, OP.add)
        dot_to_slot(r1, 1)

        # trans part: C = ohprevM.T-ish contraction, then <C, trans>
        ohp = cr.tile([128, NC2, L_], F32, name="ohp")
        nc.sync.dma_start(ohp[:],
                          d["ohprev_in"][:].rearrange("(c p) l -> p c l",
                                                      p=128))
        ohn = cr.tile([128, NC2, L_], F32, name="ohn")
        nc.sync.dma_start(ohn[:],
                          d["ohnext_in"][:].rearrange("(c p) l -> p c l",
                                                      p=128))
        pC = cps1.tile([L_, L_], F32, name="pC")
        for c2 in range(NC2):
            nc.tensor.matmul(pC[:], ohp[:, c2, :], ohn[:, c2, :],
                             start=(c2 == 0), stop=(c2 == NC2 - 1))
        tC = cr.tile([L_, L_], F32, name="tC")
        nc.vector.tensor_mul(tC[:], pC[:], trans_sb[:])
        r2 = cr.tile([L_, 1], F32, name="r2")
        nc.vector.tensor_reduce(r2[:], tC[:], AX.X, OP.add)
        dot_to_slot(r2, 2)

        # start / end parts
        oh0_sb = cr.tile([L_, B], F32, name="oh0")
        nc.sync.dma_start(oh0_sb[:], d["oh0_in"][:])
        t0 = cr.tile([L_, B], F32, name="t0")
        nc.vector.tensor_mul(t0[:], oh0_sb[:],
                             start_sb[:, :].to_broadcast([L_, B]))
        r3 = cr.tile([L_, 1], F32, name="r3")
        nc.vector.tensor_reduce(r3[:], t0[:], AX.X, OP.add)
        dot_to_slot(r3, 3)

        ohl_sb = cr.tile([L_, B], F32, name="ohl")
        nc.sync.dma_start(ohl_sb[:], d["ohlast_in"][:])
        t4 = cr.tile([L_, B], F32, name="t4")
        nc.vector.tensor_mul(t4[:], ohl_sb[:],
                             end_sb[:, :].to_broadcast([L_, B]))
        r4 = cr.tile([L_, 1], F32, name="r4")
        nc.vector.tensor_reduce(r4[:], t4[:], AX.X, OP.add)
        dot_to_slot(r4, 4)

        nc.sync.dma_start(d["out_d"][:], out_sb[:])


def _prep_core(core, sentences, mask, labels, T):
    """Per-core numpy input prep (index/layout only)."""
    NT = T * B
    NG = NT // 128
    bs = slice(B * core, B * (core + 1))
    sent = np.asarray(sentences[bs], dtype=np.int64)
    msk = np.asarray(mask[bs], dtype=bool)
    lab = np.asarray(labels[bs], dtype=np.int64)
    lens = msk.sum(axis=1).astype(np.int64)

    cols = np.arange(NT)
    tt, bb = cols // B, cols % B
    gidx = sent[bb, tt].astype(np.int32).reshape(NG, 128).T.copy()

    maskf = msk.astype(np.float32)
    lab_t = lab[bb, tt]                      # [NT]
    wmask = np.zeros((L, NT), np.float32)
    wmask[lab_t, cols] = maskf[bb, tt]

    ohprev = np.zeros((NT, L), np.float32)
    ohnext = np.zeros((NT, L), np.float32)
    valid_prev = tt >= 1
    lab_prev = lab[bb[valid_prev], tt[valid_prev] - 1]
    ohprev[cols[valid_prev], lab_prev] = maskf[bb[valid_prev],
                                               tt[valid_prev]]
    ohnext[cols, lab_t] = 1.0

    oh0 = np.zeros((L, B), np.float32)
    oh0[lab[:, 0], np.arange(B)] = 1.0
    ohlast = np.zeros((L, B), np.float32)
    ohlast[lab[np.arange(B), lens - 1], np.arange(B)] = 1.0

    lenoff = ((lens - 1).astype(np.float32) * CREB)[None, :]

    inactive = (tt == 0) | (tt >= lens[bb])
    moff = np.broadcast_to(inactive[None, :], (L, NT)).astype(np.uint8).copy()

    return {"gidx": gidx, "wmask": wmask, "ohprev": ohprev,
            "ohnext": ohnext, "oh0": oh0, "ohlast": ohlast,
            "lenoff": lenoff.astype(np.float32), "moff": moff}


def _prep_shared(emb, lstm_params, W_out, b_out, start_t, end_t, trans):
    def f32c(x):
        return np.ascontiguousarray(np.asarray(x), dtype=np.float32)

    (w1f, wh1f, bf1, w1b, wh1b, bb1), (w2f, wh2f, bf2, w2b, wh2b, bb2) = \
        lstm_params

    def bias_fm(b_):
        return np.ascontiguousarray(f32c(b_).reshape(16, 128).T)

    return {
        "emb": f32c(emb),
        "wih1f": np.ascontiguousarray(f32c(w1f).T),
        "wih1b": np.ascontiguousarray(f32c(w1b).T),
        "whh1f": np.ascontiguousarray(f32c(wh1f).T),
        "whh1b": np.ascontiguousarray(f32c(wh1b).T),
        "wih2f": np.ascontiguousarray(f32c(w2f).T),
        "wih2b": np.ascontiguousarray(f32c(w2b).T),
        "whh2f": np.ascontiguousarray(f32c(wh2f).T),
        "whh2b": np.ascontiguousarray(f32c(wh2b).T),
        "b1f": bias_fm(bf1), "b1b": bias_fm(bb1),
        "b2f": bias_fm(bf2), "b2b": bias_fm(bb2),
        "wout": np.ascontiguousarray(f32c(W_out).T),
        "bout": f32c(b_out).reshape(L, 1),
        "trans": f32c(trans),
        "start": f32c(start_t).reshape(L, 1),
        "end": f32c(end_t).reshape(L, 1),
    }


def run(sentences, mask, labels, emb, lstm_params, W_out, b_out, start_t,
        end_t, trans, T=None, V=None):
    T = T if T is not None else np.asarray(sentences).shape[1]
    V = V if V is not None else np.asarray(emb).shape[0]
    import os
    key = (T, V, os.environ.get("BASS_PHASES", "6"))
    if key not in _CACHE:
        _CACHE[key] = _build(T, V)
    nc = _CACHE[key]

    shared = _prep_shared(emb, lstm_params, W_out, b_out, start_t, end_t,
                          trans)
    in_maps = []
    for core in range(8):
        m = dict(shared)
        m.update(_prep_core(core, sentences, mask, labels, T))
        in_maps.append(m)
    res = run_bass_kernel_spmd(nc, in_maps, list(range(8)))
    total = 0.0
    for core in range(8):
        o = res.results[core]["out"][0]
        total += float(o[0]) - float(o[1] + o[2] + o[3] + o[4])
    return np.float32(total / 64.0)


def kernel(sentences, mask, labels, emb, lstm_params, W_out, b_out, start_t,
           end_t, trans):
    return run(sentences, mask, labels, emb, lstm_params, W_out, b_out,
               start_t, end_t, trans)


# revision 20
# speedup vs baseline: 1.0097x; 1.0097x over previous
"""BiLSTM-CRF forward loss on 8 Trainium2 NeuronCores.

Strategy: pure data-parallel over batch (8 sequences per core, no
cross-core communication).  Per core:
  1. embedding gather (indirect DMA) + PE-transpose -> x.T feature-major
  2. input-projection GEMMs (fp32r, full rate at N=512) -> u = x @ Wih.T + b
  3. two BiLSTM layers: forward+backward scans interleaved on the same
     core; recurrent matmuls in bf16 with weights stationary
     (feature-major h, no transposes in the loop)
  4. emission GEMM (fp32r) -> emissions feature-major [17, T*8]
  5. CRF partition function via a blocked exp-domain scan (8 parallel
     chunk products, log-domain combine) + numerator via masked one-hot
     contractions.  Each core returns 5 scalars; the host combines them.
"""

import sys

for _p in ("/opt/trn_rl_repo", "/root/.axon_site/_ro/trn_rl_repo"):
    if _p not in sys.path:
        sys.path.insert(0, _p)

import numpy as np

import concourse.bass as bass
import concourse.bacc as bacc
import concourse.mybir as mybir
import concourse.tile as tile
from concourse.bass import ds, ts
from concourse.bass_utils import run_bass_kernel_spmd
from concourse.masks import make_identity

F32 = mybir.dt.float32
F32R = mybir.dt.float32r
BF16 = mybir.dt.bfloat16
I32 = mybir.dt.int32
U8 = mybir.dt.uint8
AF = mybir.ActivationFunctionType
OP = mybir.AluOpType
AX = mybir.AxisListType

B = 8            # sequences per core
E = 512          # embedding dim
H = 512          # hidden per direction
G = 2048         # 4*H gate rows
L = 17           # number of tags
CREB = 2.83      # CRF exp-domain rebase constant (log-domain growth/step)
CC = 8           # CRF chunk count

_CACHE = {}


def _build(T, V):
    NT = T * B                 # tokens per core, time-major cols (t*B + b)
    NG = NT // 128             # gather tiles of 128 tokens
    NCH = T // 16              # scan chunks of 16 timesteps
    W = min(512, NT)           # GEMM window (moving free dim)
    NW = NT // W
    CL = T // CC               # CRF chunk length
    NC2 = NT // 128            # 128-col chunks for the trans-count matmul

    nc = bacc.Bacc(None, target_bir_lowering=False, debug=False)

    # ---------------- kernel I/O ----------------
    emb_in = nc.dram_tensor("emb", [V, E], F32, kind="ExternalInput")
    gidx_in = nc.dram_tensor("gidx", [128, NG], I32, kind="ExternalInput")
    wih1f_in = nc.dram_tensor("wih1f", [E, G], F32, kind="ExternalInput")
    wih1b_in = nc.dram_tensor("wih1b", [E, G], F32, kind="ExternalInput")
    whh1f_in = nc.dram_tensor("whh1f", [H, G], F32, kind="ExternalInput")
    whh1b_in = nc.dram_tensor("whh1b", [H, G], F32, kind="ExternalInput")
    wih2f_in = nc.dram_tensor("wih2f", [2 * H, G], F32, kind="ExternalInput")
    wih2b_in = nc.dram_tensor("wih2b", [2 * H, G], F32, kind="ExternalInput")
    whh2f_in = nc.dram_tensor("whh2f", [H, G], F32, kind="ExternalInput")
    whh2b_in = nc.dram_tensor("whh2b", [H, G], F32, kind="ExternalInput")
    b1f_in = nc.dram_tensor("b1f", [128, 16], F32, kind="ExternalInput")
    b1b_in = nc.dram_tensor("b1b", [128, 16], F32, kind="ExternalInput")
    b2f_in = nc.dram_tensor("b2f", [128, 16], F32, kind="ExternalInput")
    b2b_in = nc.dram_tensor("b2b", [128, 16], F32, kind="ExternalInput")
    wout_in = nc.dram_tensor("wout", [2 * H, L], F32, kind="ExternalInput")
    bout_in = nc.dram_tensor("bout", [L, 1], F32, kind="ExternalInput")
    trans_in = nc.dram_tensor("trans", [L, L], F32, kind="ExternalInput")
    start_in = nc.dram_tensor("start", [L, 1], F32, kind="ExternalInput")
    end_in = nc.dram_tensor("end", [L, 1], F32, kind="ExternalInput")
    wmask_in = nc.dram_tensor("wmask", [L, NT], F32, kind="ExternalInput")
    ohprev_in = nc.dram_tensor("ohprev", [NT, L], F32, kind="ExternalInput")
    ohnext_in = nc.dram_tensor("ohnext", [NT, L], F32, kind="ExternalInput")
    oh0_in = nc.dram_tensor("oh0", [L, B], F32, kind="ExternalInput")
    ohlast_in = nc.dram_tensor("ohlast", [L, B], F32, kind="ExternalInput")
    lenoff_in = nc.dram_tensor("lenoff", [1, B], F32, kind="ExternalInput")
    moff_in = nc.dram_tensor("moff", [L, NT], U8, kind="ExternalInput")

    out_d = nc.dram_tensor("out", [1, 8], F32, kind="ExternalOutput")

    # ---------------- internal DRAM ----------------
    u1f_d = nc.dram_tensor("u1f", [16, 128, NT], BF16)
    u1b_d = nc.dram_tensor("u1b", [16, 128, NT], BF16)
    u2f_d = nc.dram_tensor("u2f", [16, 128, NT], BF16)
    u2b_d = nc.dram_tensor("u2b", [16, 128, NT], BF16)
    h1_d = nc.dram_tensor("h1", [8, 128, NT], BF16)  # rows 0-3 fwd, 4-7 bwd
    h2_d = nc.dram_tensor("h2", [8, 128, NT], BF16)

    with tile.TileContext(nc) as tc:
        _emit(tc, locals(), T=T, V=V, NT=NT, NG=NG, NCH=NCH, W=W, NW=NW,
              CL=CL, NC2=NC2)
    nc.compile()
    return nc


def _emit(tc, d, *, T, V, NT, NG, NCH, W, NW, CL, NC2):
    import os
    PH = int(os.environ.get("BASS_PHASES", "6"))
    nc = tc.nc

    # persistent pool for things that live across phases
    with tc.tile_pool(name="persist", bufs=1) as perst, \
         tc.tile_pool(name="ps_persist", bufs=1, space="PSUM") as psper:

        ident = perst.tile([128, 128], F32)
        make_identity(nc, ident[:])

        # =========== phase 0: gather + transpose -> xT (f32r) ===========
        with tc.tile_pool(name="xtp", bufs=1) as xtp:
            xT = xtp.tile([128, 4, NT], BF16, name="xT")
            with tc.tile_pool(name="p0", bufs=3) as p0, \
                 tc.tile_pool(name="ps0", bufs=4, space="PSUM") as ps0:
                gidx_sb = p0.tile([128, NG], I32, name="gidx")
                nc.sync.dma_start(gidx_sb[:], d["gidx_in"][:])
                for j in range(NG):
                    gx = p0.tile([128, E], F32, name="gx")
                    nc.gpsimd.indirect_dma_start(
                        out=gx[:], out_offset=None, in_=d["emb_in"][:],
                        in_offset=bass.IndirectOffsetOnAxis(
                            ap=gidx_sb[:, j:j + 1], axis=0))
                    for k in range(4):
                        pst = ps0.tile([128, 128], F32, name="ptr")
                        nc.tensor.transpose(pst[:],
                                            gx[:, 128 * k:128 * (k + 1)],
                                            ident[:])
                        nc.vector.tensor_copy(
                            xT[:, k, 128 * j:128 * (j + 1)], pst[:])

            # =========== phase 1: layer-1 input GEMMs ===========
            if PH >= 1:
                _ugemm(tc, d["wih1f_in"], d["b1f_in"], d["u1f_d"], 4, NT, W,
                       NW, rhs_sbuf=xT)
                _ugemm(tc, d["wih1b_in"], d["b1b_in"], d["u1b_d"], 4, NT, W,
                       NW, rhs_sbuf=xT)

        # =========== phase 2: layer-1 scans ===========
        if PH >= 2:
            _scan(tc, d["u1f_d"], d["u1b_d"], d["whh1f_in"], d["whh1b_in"],
                  d["h1_d"], NT, NCH)

        # =========== phase 3: layer-2 input GEMMs ===========
        if PH >= 3:
            _ugemm(tc, d["wih2f_in"], d["b2f_in"], d["u2f_d"], 8, NT, W, NW,
                   rhs_dram=d["h1_d"])
            _ugemm(tc, d["wih2b_in"], d["b2b_in"], d["u2b_d"], 8, NT, W, NW,
                   rhs_dram=d["h1_d"])

        # =========== phase 4: layer-2 scans ===========
        if PH >= 4:
            _scan(tc, d["u2f_d"], d["u2b_d"], d["whh2f_in"], d["whh2b_in"],
                  d["h2_d"], NT, NCH)

        # =========== phase 5: emissions ===========
        if PH < 5:
            with tc.tile_pool(name="stub", bufs=1) as stub:
                zo = stub.tile([1, 8], F32, name="zo")
                nc.vector.memset(zo[:], 0.0)
                nc.sync.dma_start(d["out_d"][:], zo[:])
            return
        em = perst.tile([L, NT], F32, name="em")
        with tc.tile_pool(name="pe", bufs=3) as pe, \
             tc.tile_pool(name="pse", bufs=4, space="PSUM") as pse:
            wo_st = pe.tile([128, 8, L], F32, name="wo_st")
            nc.sync.dma_start(
                wo_st[:], d["wout_in"][:].rearrange("(k p) l -> p k l", p=128))
            wo = pe.tile([128, 8, L], BF16, name="wo")
            nc.vector.tensor_copy(wo[:], wo_st[:])
            bout_sb = pe.tile([L, 1], F32, name="bout")
            nc.sync.dma_start(bout_sb[:], d["bout_in"][:])
            for w in range(NW):
                rh = pe.tile([128, 8, W], BF16, name="rh")
                nc.sync.dma_start(
                    rh[:],
                    d["h2_d"][:].rearrange("k p n -> p k n")[:, :,
                                                            ts(w, W)])
                pst = pse.tile([L, W], F32, name="pem")
                for k in range(8):
                    nc.tensor.matmul(pst[:], wo[:, k, :], rh[:, k, :],
                                     start=(k == 0), stop=(k == 7))
                nc.vector.tensor_add(
                    em[:, ts(w, W)], pst[:],
                    bout_sb[:, :].to_broadcast([L, W]))

        # =========== phase 6: CRF ===========
        if PH >= 6:
            _crf(tc, d, em, perst, psper, T, NT, CL, NC2)
        else:
            with tc.tile_pool(name="stub", bufs=1) as stub:
                zo = stub.tile([1, 8], F32, name="zo")
                nc.vector.memset(zo[:], 0.0)
                nc.sync.dma_start(d["out_d"][:], zo[:])


def _ugemm(tc, w_dram, b_dram, u_dram, KC, NT, W, NW, rhs_sbuf=None,
           rhs_dram=None):
    """u.T[m-tile] = sum_k Wih.T[k,mtile].T @ rhs[k]  (+ bias), fp32r."""
    nc = tc.nc
    with tc.tile_pool(name="ug", bufs=2) as ug, \
         tc.tile_pool(name="ug_w", bufs=1) as ugw, \
         tc.tile_pool(name="ug_ps", bufs=4, space="PSUM") as ups:
        wt = ugw.tile([128, KC, 2048], BF16, name="wt")
        for k in range(KC):
            st = ug.tile([128, 2048], F32, name="wstage")
            nc.sync.dma_start(st[:], w_dram[ds(128 * k, 128), :])
            nc.vector.tensor_copy(wt[:, k, :], st[:])
        bias = ugw.tile([128, 16], F32, name="bias")
        nc.sync.dma_start(bias[:], b_dram[:])
        for w in range(NW):
            if rhs_sbuf is not None:
                rh = rhs_sbuf[:, :, ts(w, W)]
            else:
                rh_t = ug.tile([128, KC, W], BF16, name="rh_t")
                nc.sync.dma_start(
                    rh_t[:],
                    rhs_dram[:].rearrange("k p n -> p k n")[:, :, ts(w, W)])
                rh = rh_t[:]
            for m in range(16):
                pst = ups.tile([128, W], F32, name="pu")
                for k in range(KC):
                    nc.tensor.matmul(pst[:], wt[:, k, 128 * m:128 * (m + 1)],
                                     rh[:, k, :],
                                     start=(k == 0), stop=(k == KC - 1))
                usb = ug.tile([128, W], BF16, name="usb")
                nc.vector.tensor_add(
                    usb[:], pst[:],
                    bias[:, m:m + 1].to_broadcast([128, W]))
                nc.sync.dma_start(u_dram[m, :, ts(w, W)], usb[:])


def _scan(tc, uf_dram, ub_dram, whf_dram, whb_dram, hT_dram, NT, NCH):
    """Interleaved fwd+bwd LSTM scans, bf16 recurrent matmuls."""
    nc = tc.nc
    with tc.tile_pool(name="sc_w", bufs=1) as scw, \
         tc.tile_pool(name="sc_st", bufs=1) as scs, \
         tc.tile_pool(name="sc", bufs=3) as sc, \
         tc.tile_pool(name="sc_ps", bufs=4, space="PSUM") as sps:
        wbf = []
        with tc.tile_pool(name="sc_wst", bufs=1) as scst:
            for wi, w_dram in enumerate((whf_dram, whb_dram)):
                st = scst.tile([128, 4, 2048], F32, name=f"whstage{wi}",
                               tag="whstage")
                nc.sync.dma_start(
                    st[:], w_dram[:].rearrange("(k p) m -> p k m", p=128))
                wb = scw.tile([128, 4, 2048], BF16, name=f"whbf{wi}",
                              tag=f"whbf{wi}")
                nc.vector.tensor_copy(wb[:], st[:])
                wbf.append(wb)

        hst = [scs.tile([128, 4, B], BF16, name=f"h_{i}") for i in range(2)]
        cst = [scs.tile([128, 4, B], F32, name=f"c_{i}") for i in range(2)]
        for t_ in hst + cst:
            nc.vector.memset(t_[:], 0.0)

        uf_ap = uf_dram[:].rearrange("m p n -> p m n")
        ub_ap = ub_dram[:].rearrange("m p n -> p m n")
        hT_f = hT_dram[ds(0, 4)].rearrange("k p n -> p k n")
        hT_b = hT_dram[ds(4, 4)].rearrange("k p n -> p k n")

        with tc.For_i(0, NCH, hint_engines=(mybir.EngineType.PE,),
                      name="scan") as ci:
            u_sb = []
            for di, u_ap in enumerate((uf_ap, ub_ap)):
                ut = sc.tile([128, 16, 128], BF16, name=f"u_{di}")
                if di == 0:
                    nc.sync.dma_start(ut[:], u_ap[:, :, ts(ci, 128)])
                else:
                    nc.sync.dma_start(
                        ut[:], u_ap[:, :, ds(ci * (-128) + (NT - 128), 128)])
                u_sb.append(ut)
            hout = [sc.tile([128, 4, 128], BF16, name=f"ho_{di}")
                    for di in range(2)]
            for j in range(16):
                for di in range(2):
                    h_bf, c_sb = hst[di], cst[di]
                    col = 8 * j if di == 0 else 8 * (15 - j)
                    pst = sps.tile([128, 128], F32, name=f"pg_{di}")
                    for m in range(16):
                        for k in range(4):
                            nc.tensor.matmul(
                                pst[:, 8 * m:8 * (m + 1)],
                                wbf[di][:, k, 128 * m:128 * (m + 1)],
                                h_bf[:, k, :],
                                start=(k == 0), stop=(k == 3))
                    g = sc.tile([128, 128], F32, name=f"g_{di}")
                    nc.vector.tensor_add(
                        g[:].rearrange("p (m b) -> p m b", m=16),
                        pst[:].rearrange("p (m b) -> p m b", m=16),
                        u_sb[di][:, :, ds(col, 8)])
                    sif = sc.tile([128, 64], F32, name=f"sif_{di}")
                    nc.scalar.activation(sif[:], g[:, 0:64], AF.Sigmoid)
                    tg = sc.tile([128, 32], F32, name=f"tg_{di}")
                    nc.scalar.activation(tg[:], g[:, 64:96], AF.Tanh)
                    so = sc.tile([128, 32], F32, name=f"so_{di}")
                    nc.scalar.activation(so[:], g[:, 96:128], AF.Sigmoid)
                    t1 = sc.tile([128, 32], F32, name=f"t1_{di}")
                    nc.vector.tensor_mul(
                        t1[:], sif[:, 32:64],
                        c_sb[:].rearrange("p k b -> p (k b)"))
                    t2 = sc.tile([128, 32], F32, name=f"t2_{di}")
                    nc.vector.tensor_mul(t2[:], sif[:, 0:32], tg[:])
                    nc.vector.tensor_add(
                        c_sb[:].rearrange("p k b -> p (k b)"), t1[:], t2[:])
                    tcn = sc.tile([128, 32], F32, name=f"tc_{di}")
                    nc.scalar.activation(
                        tcn[:], c_sb[:].rearrange("p k b -> p (k b)"),
                        AF.Tanh)
                    nc.vector.tensor_mul(
                        hout[di][:, :, ds(col, 8)],
                        so[:].rearrange("p (k b) -> p k b", k=4),
                        tcn[:].rearrange("p (k b) -> p k b", k=4))
                    nc.vector.tensor_mul(
                        h_bf[:].rearrange("p k b -> p (k b)"),
                        so[:], tcn[:])
            nc.sync.dma_start(hT_f[:, :, ts(ci, 128)], hout[0][:])
            nc.sync.dma_start(hT_b[:, :, ds(ci * (-128) + (NT - 128), 128)],
                              hout[1][:])


def _crf(tc, d, em, perst, psper, T, NT, CL, NC2):
    nc = tc.nc
    L_ = L
    with tc.tile_pool(name="cr", bufs=2) as cr, \
         tc.tile_pool(name="cr_st", bufs=1) as crs, \
         tc.tile_pool(name="cr_ps", bufs=2, space="PSUM") as cps, \
         tc.tile_pool(name="cr_ps1", bufs=1, space="PSUM") as cps1:
        # constants
        trans_sb = crs.tile([L_, L_], F32, name="trans")
        nc.sync.dma_start(trans_sb[:], d["trans_in"][:])
        P_sb = crs.tile([L_, L_], F32, name="P")
        nc.scalar.activation(P_sb[:], trans_sb[:], AF.Exp)
        start_sb = crs.tile([L_, 1], F32, name="start")
        nc.sync.dma_start(start_sb[:], d["start_in"][:])
        end_sb = crs.tile([L_, 1], F32, name="end")
        nc.sync.dma_start(end_sb[:], d["end_in"][:])
        id17 = crs.tile([L_, L_], F32, name="id17")
        make_identity(nc, id17[:])
        one17 = crs.tile([L_, 1], F32, name="one17")
        nc.vector.memset(one17[:], 1.0)
        onerow = crs.tile([1, L_], F32, name="onerow")
        nc.vector.memset(onerow[:], 1.0)
        moff_sb = crs.tile([L_, NT], U8, name="moff")
        nc.sync.dma_start(moff_sb[:], d["moff_in"][:])
        lenoff_sb = crs.tile([1, B], F32, name="lenoff")
        nc.sync.dma_start(lenoff_sb[:], d["lenoff_in"][:])
        out_sb = crs.tile([1, 8], F32, name="outv")
        nc.vector.memset(out_sb[:], 0.0)

        # Emask = exp(em - CREB), 1.0 where step inactive
        crebt = crs.tile([L_, 1], F32, name="crebt")
        nc.vector.memset(crebt[:], -CREB)
        emask = perst.tile([L_, NT], F32, name="emask")
        nc.scalar.activation(emask[:], em[:], AF.Exp, bias=crebt[:, :])
        nc.vector.copy_predicated(emask[:], moff_sb[:],
                                  one17[:, :].to_broadcast([L_, NT]))

        # ---- blocked exp-domain chunk scan ----
        # state At[k, (c, b, i)] = (prod_{tau} P diag(E))^T per chunk c
        NFREE = CC * B * L_
        A = [crs.tile([L_, CC, B, L_], F32, name=f"A{i}") for i in range(2)]
        nc.vector.tensor_copy(
            A[0][:],
            id17[:, None, None, :].to_broadcast([L_, CC, B, L_]))
        NGRP = 4
        CG = CC // NGRP          # chunks per matmul group
        GW = CG * B * L_         # 272
        for tau in range(CL):
            src, dst = A[tau % 2], A[(tau + 1) % 2]
            emsl = emask[:].rearrange("p (c s b) -> p c s b", c=CC,
                                      s=CL)[:, :, tau, :]
            mosl = moff_sb[:].rearrange("p (c s b) -> p c s b", c=CC,
                                        s=CL)[:, :, tau, :]
            for gp in range(NGRP):
                cs = slice(CG * gp, CG * (gp + 1))
                pA = cps.tile([L_, GW], F32, name="pA")
                nc.tensor.matmul(
                    pA[:], P_sb[:],
                    src[:].rearrange("p c b i -> p (c b i)")[:, ts(gp, GW)],
                    start=True, stop=True)
                nc.vector.tensor_mul(
                    dst[:, cs, :, :],
                    pA[:].rearrange("p (c b i) -> p c b i", c=CG, b=B),
                    emsl[:, cs, :, None].to_broadcast([L_, CG, B, L_]))
            nc.vector.copy_predicated(
                dst[:],
                mosl[:, :, :, None].to_broadcast([L_, CC, B, L_]),
                src[:])
        Afin = A[CL % 2]
        logA = perst.tile([L_, CC, B, L_], F32, name="logA")
        nc.scalar.activation(logA[:], Afin[:], AF.Ln)
        nc.vector.tensor_scalar_max(logA[:], logA[:], -1e30)

        # ---- log-domain combine ----
        s_fm = crs.tile([L_, B], F32, name="s_fm")   # s[j, b]
        nc.vector.tensor_add(s_fm[:], em[:, 0:B],
                             start_sb[:, :].to_broadcast([L_, B]))
        s_rep = crs.tile([L_, B, L_], F32, name="s_rep")

        def replicate(src_fm):
            # s_rep[k, (b, i)] = src_fm[i, b] for all k
            pT = cps1.tile([B, L_], F32, name="pT")
            nc.tensor.transpose(pT[:], src_fm[:], id17[:])
            sT = cr.tile([B, L_], F32, name="sT")
            nc.vector.tensor_copy(sT[:], pT[:])
            srow = cr.tile([1, B * L_], F32, name="srow")
            nc.sync.dma_start(srow[:], sT[:])
            pR = cps1.tile([L_, B * L_], F32, name="pR")
            nc.tensor.matmul(pR[:], onerow[:], srow[:], start=True, stop=True)
            nc.vector.tensor_copy(
                s_rep[:], pR[:].rearrange("p (b i) -> p b i", b=B))

        replicate(s_fm)
        for c in range(CC):
            tmp = cr.tile([L_, B, L_], F32, name="ctmp")
            nc.vector.tensor_add(tmp[:], logA[:, c, :, :], s_rep[:])
            etmp = cr.tile([L_, B, L_], F32, name="cetmp")
            nc.scalar.activation(etmp[:], tmp[:], AF.Exp)
            sex = cr.tile([L_, B], F32, name="sex")
            nc.vector.tensor_reduce(sex[:], etmp[:], AX.X, OP.add)
            nc.scalar.activation(s_fm[:], sex[:], AF.Ln)
            if c < CC - 1:
                replicate(s_fm)

        # logZ_b = LSE_k(s[k,b] + end[k]) + lenoff[b]; out slot0 = sum_b
        send = cr.tile([L_, B], F32, name="send")
        nc.vector.tensor_add(send[:], s_fm[:],
                             end_sb[:, :].to_broadcast([L_, B]))
        eend = cr.tile([L_, B], F32, name="eend")
        nc.scalar.activation(eend[:], send[:], AF.Exp)
        pz = cps1.tile([1, B], F32, name="pz")
        nc.tensor.matmul(pz[:], one17[:], eend[:], start=True, stop=True)
        lz = cr.tile([1, B], F32, name="lz")
        nc.scalar.activation(lz[:], pz[:], AF.Ln)
        nc.vector.tensor_add(lz[:], lz[:], lenoff_sb[:])
        nc.vector.tensor_reduce(out_sb[:, 0:1], lz[:], AX.X, OP.add)

        # ---- numerator parts ----
        def dot_to_slot(vec_lp, slot):
            # vec_lp: [L, 1] -> sum over partitions into out_sb[0, slot]
            pd = cps1.tile([1, 1], F32, name="pd")
            nc.tensor.matmul(pd[:], one17[:], vec_lp[:], start=True,
                             stop=True)
            nc.vector.tensor_copy(out_sb[:, slot:slot + 1], pd[:])

        # e_tag part
        wm_sb = cr.tile([L_, NT], F32, name="wm")
        nc.sync.dma_start(wm_sb[:], d["wmask_in"][:])
        prod = cr.tile([L_, NT], F32, name="prod")
        nc.vector.tensor_mul(prod[:], em[:], wm_sb[:])
        r1 = cr.tile([L_, 1], F32, name="r1")
        nc.vector.tensor_reduce(r1[:], prod[:], AX.X, OP.add)
        dot_to_slot(r1, 1)

        # trans part: C = ohprevM.T-ish contraction, then <C, trans>
        ohp = cr.tile([128, NC2, L_], F32, name="ohp")
        nc.sync.dma_start(ohp[:],
                          d["ohprev_in"][:].rearrange("(c p) l -> p c l",
                                                      p=128))
        ohn = cr.tile([128, NC2, L_], F32, name="ohn")
        nc.sync.dma_start(ohn[:],
                          d["ohnext_in"][:].rearrange("(c p) l -> p c l",
                                                      p=128))
        pC = cps1.tile([L_, L_], F32, name="pC")
        for c2 in range(NC2):
            nc.tensor.matmul(pC[:], ohp[:, c2, :], ohn[:, c2, :],
                             start=(c2 == 0), stop=(c2 == NC2 - 1))
        tC = cr.tile([L_, L_], F32, name="tC")
        nc.vector.tensor_mul(tC[:], pC[:], trans_sb[:])
        r2 = cr.tile([L_, 1], F32, name="r2")
        nc.vector.tensor_reduce(r2[:], tC[:], AX.X, OP.add)
        dot_to_slot(r2, 2)

        # start / end parts
        oh0_sb = cr.tile([L_, B], F32, name="oh0")
        nc.sync.dma_start(oh0_sb[:], d["oh0_in"][:])
        t0 = cr.tile([L_, B], F32, name="t0")
        nc.vector.tensor_mul(t0[:], oh0_sb[:],
                             start_sb[:, :].to_broadcast([L_, B]))
        r3 = cr.tile([L_, 1], F32, name="r3")
        nc.vector.tensor_reduce(r3[:], t0[:], AX.X, OP.add)
        dot_to_slot(r3, 3)

        ohl_sb = cr.tile([L_, B], F32, name="ohl")
        nc.sync.dma_start(ohl_sb[:], d["ohlast_in"][:])
        t4 = cr.tile([L_, B], F32, name="t4")
        nc.vector.tensor_mul(t4[:], ohl_sb[:],
                             end_sb[:, :].to_broadcast([L_, B]))
        r4 = cr.tile([L_, 1], F32, name="r4")
        nc.vector.tensor_reduce(r4[:], t4[:], AX.X, OP.add)
        dot_to_slot(r4, 4)

        nc.sync.dma_start(d["out_d"][:], out_sb[:])


def _prep_core(core, sentences, mask, labels, T):
    """Per-core numpy input prep (index/layout only)."""
    NT = T * B
    NG = NT // 128
    bs = slice(B * core, B * (core + 1))
    sent = np.asarray(sentences[bs], dtype=np.int64)
    msk = np.asarray(mask[bs], dtype=bool)
    lab = np.asarray(labels[bs], dtype=np.int64)
    lens = msk.sum(axis=1).astype(np.int64)

    cols = np.arange(NT)
    tt, bb = cols // B, cols % B
    gidx = sent[bb, tt].astype(np.int32).reshape(NG, 128).T.copy()

    maskf = msk.astype(np.float32)
    lab_t = lab[bb, tt]                      # [NT]
    wmask = np.zeros((L, NT), np.float32)
    wmask[lab_t, cols] = maskf[bb, tt]

    ohprev = np.zeros((NT, L), np.float32)
    ohnext = np.zeros((NT, L), np.float32)
    valid_prev = tt >= 1
    lab_prev = lab[bb[valid_prev], tt[valid_prev] - 1]
    ohprev[cols[valid_prev], lab_prev] = maskf[bb[valid_prev],
                                               tt[valid_prev]]
    ohnext[cols, lab_t] = 1.0

    oh0 = np.zeros((L, B), np.float32)
    oh0[lab[:, 0], np.arange(B)] = 1.0
    ohlast = np.zeros((L, B), np.float32)
    ohlast[lab[np.arange(B), lens - 1], np.arange(B)] = 1.0

    lenoff = ((lens - 1).astype(np.float32) * CREB)[None, :]

    inactive = (tt == 0) | (tt >= lens[bb])
    moff = np.broadcast_to(inactive[None, :], (L, NT)).astype(np.uint8).copy()

    return {"gidx": gidx, "wmask": wmask, "ohprev": ohprev,
            "ohnext": ohnext, "oh0": oh0, "ohlast": ohlast,
            "lenoff": lenoff.astype(np.float32), "moff": moff}


def _prep_shared(emb, lstm_params, W_out, b_out, start_t, end_t, trans):
    def f32c(x):
        return np.ascontiguousarray(np.asarray(x), dtype=np.float32)

    (w1f, wh1f, bf1, w1b, wh1b, bb1), (w2f, wh2f, bf2, w2b, wh2b, bb2) = \
        lstm_params

    def bias_fm(b_):
        return np.ascontiguousarray(f32c(b_).reshape(16, 128).T)

    return {
        "emb": f32c(emb),
        "wih1f": np.ascontiguousarray(f32c(w1f).T),
        "wih1b": np.ascontiguousarray(f32c(w1b).T),
        "whh1f": np.ascontiguousarray(f32c(wh1f).T),
        "whh1b": np.ascontiguousarray(f32c(wh1b).T),
        "wih2f": np.ascontiguousarray(f32c(w2f).T),
        "wih2b": np.ascontiguousarray(f32c(w2b).T),
        "whh2f": np.ascontiguousarray(f32c(wh2f).T),
        "whh2b": np.ascontiguousarray(f32c(wh2b).T),
        "b1f": bias_fm(bf1), "b1b": bias_fm(bb1),
        "b2f": bias_fm(bf2), "b2b": bias_fm(bb2),
        "wout": np.ascontiguousarray(f32c(W_out).T),
        "bout": f32c(b_out).reshape(L, 1),
        "trans": f32c(trans),
        "start": f32c(start_t).reshape(L, 1),
        "end": f32c(end_t).reshape(L, 1),
    }


def run(sentences, mask, labels, emb, lstm_params, W_out, b_out, start_t,
        end_t, trans, T=None, V=None):
    T = T if T is not None else np.asarray(sentences).shape[1]
    V = V if V is not None else np.asarray(emb).shape[0]
    import os
    key = (T, V, os.environ.get("BASS_PHASES", "6"))
    if key not in _CACHE:
        _CACHE[key] = _build(T, V)
    nc = _CACHE[key]

    shared = _prep_shared(emb, lstm_params, W_out, b_out, start_t, end_t,
                          trans)
    in_maps = []
    for core in range(8):
        m = dict(shared)
        m.update(_prep_core(core, sentences, mask, labels, T))
        in_maps.append(m)
    res = run_bass_kernel_spmd(nc, in_maps, list(range(8)))
    total = 0.0
    for core in range(8):
        o = res.results[core]["out"][0]
        total += float(o[0]) - float(o[1] + o[2] + o[3] + o[4])
    return np.float32(total / 64.0)


def kernel(sentences, mask, labels, emb, lstm_params, W_out, b_out, start_t,
           end_t, trans):
    return run(sentences, mask, labels, emb, lstm_params, W_out, b_out,
               start_t, end_t, trans)


# revision 21
# speedup vs baseline: 1.0332x; 1.0234x over previous
"""BiLSTM-CRF forward loss on 8 Trainium2 NeuronCores.

Strategy: pure data-parallel over batch (8 sequences per core, no
cross-core communication).  Per core:
  1. embedding gather (indirect DMA) + PE-transpose -> x.T feature-major
  2. input-projection GEMMs (fp32r, full rate at N=512) -> u = x @ Wih.T + b
  3. two BiLSTM layers: forward+backward scans interleaved on the same
     core; recurrent matmuls in bf16 with weights stationary
     (feature-major h, no transposes in the loop)
  4. emission GEMM (fp32r) -> emissions feature-major [17, T*8]
  5. CRF partition function via a blocked exp-domain scan (8 parallel
     chunk products, log-domain combine) + numerator via masked one-hot
     contractions.  Each core returns 5 scalars; the host combines them.
"""

import sys

for _p in ("/opt/trn_rl_repo", "/root/.axon_site/_ro/trn_rl_repo"):
    if _p not in sys.path:
        sys.path.insert(0, _p)

import numpy as np

import concourse.bass as bass
import concourse.bacc as bacc
import concourse.mybir as mybir
import concourse.tile as tile
from concourse.bass import ds, ts
from concourse.bass_utils import run_bass_kernel_spmd
from concourse.masks import make_identity

F32 = mybir.dt.float32
F32R = mybir.dt.float32r
BF16 = mybir.dt.bfloat16
I32 = mybir.dt.int32
U8 = mybir.dt.uint8
AF = mybir.ActivationFunctionType
OP = mybir.AluOpType
AX = mybir.AxisListType

B = 8            # sequences per core
E = 512          # embedding dim
H = 512          # hidden per direction
G = 2048         # 4*H gate rows
L = 17           # number of tags
CREB = 2.83      # CRF exp-domain rebase constant (log-domain growth/step)
CC = 8           # CRF chunk count

_CACHE = {}


def _build(T, V):
    NT = T * B                 # tokens per core, time-major cols (t*B + b)
    NG = NT // 128             # gather tiles of 128 tokens
    NCH = T // 16              # scan chunks of 16 timesteps
    W = min(512, NT)           # GEMM window (moving free dim)
    NW = NT // W
    CL = T // CC               # CRF chunk length
    NC2 = NT // 128            # 128-col chunks for the trans-count matmul

    nc = bacc.Bacc(None, target_bir_lowering=False, debug=False,
                   num_swdge_queues=4)

    # ---------------- kernel I/O ----------------
    emb_in = nc.dram_tensor("emb", [V, E], F32, kind="ExternalInput")
    gidx_in = nc.dram_tensor("gidx", [128, NG], I32, kind="ExternalInput")
    wih1f_in = nc.dram_tensor("wih1f", [E, G], F32, kind="ExternalInput")
    wih1b_in = nc.dram_tensor("wih1b", [E, G], F32, kind="ExternalInput")
    whh1f_in = nc.dram_tensor("whh1f", [H, G], F32, kind="ExternalInput")
    whh1b_in = nc.dram_tensor("whh1b", [H, G], F32, kind="ExternalInput")
    wih2f_in = nc.dram_tensor("wih2f", [2 * H, G], F32, kind="ExternalInput")
    wih2b_in = nc.dram_tensor("wih2b", [2 * H, G], F32, kind="ExternalInput")
    whh2f_in = nc.dram_tensor("whh2f", [H, G], F32, kind="ExternalInput")
    whh2b_in = nc.dram_tensor("whh2b", [H, G], F32, kind="ExternalInput")
    b1f_in = nc.dram_tensor("b1f", [128, 16], F32, kind="ExternalInput")
    b1b_in = nc.dram_tensor("b1b", [128, 16], F32, kind="ExternalInput")
    b2f_in = nc.dram_tensor("b2f", [128, 16], F32, kind="ExternalInput")
    b2b_in = nc.dram_tensor("b2b", [128, 16], F32, kind="ExternalInput")
    wout_in = nc.dram_tensor("wout", [2 * H, L], F32, kind="ExternalInput")
    bout_in = nc.dram_tensor("bout", [L, 1], F32, kind="ExternalInput")
    trans_in = nc.dram_tensor("trans", [L, L], F32, kind="ExternalInput")
    start_in = nc.dram_tensor("start", [L, 1], F32, kind="ExternalInput")
    end_in = nc.dram_tensor("end", [L, 1], F32, kind="ExternalInput")
    wmask_in = nc.dram_tensor("wmask", [L, NT], F32, kind="ExternalInput")
    ohprev_in = nc.dram_tensor("ohprev", [NT, L], F32, kind="ExternalInput")
    ohnext_in = nc.dram_tensor("ohnext", [NT, L], F32, kind="ExternalInput")
    oh0_in = nc.dram_tensor("oh0", [L, B], F32, kind="ExternalInput")
    ohlast_in = nc.dram_tensor("ohlast", [L, B], F32, kind="ExternalInput")
    lenoff_in = nc.dram_tensor("lenoff", [1, B], F32, kind="ExternalInput")
    moff_in = nc.dram_tensor("moff", [L, NT], U8, kind="ExternalInput")

    out_d = nc.dram_tensor("out", [1, 8], F32, kind="ExternalOutput")

    # ---------------- internal DRAM ----------------
    u1f_d = nc.dram_tensor("u1f", [16, 128, NT], BF16)
    u1b_d = nc.dram_tensor("u1b", [16, 128, NT], BF16)
    u2f_d = nc.dram_tensor("u2f", [16, 128, NT], BF16)
    u2b_d = nc.dram_tensor("u2b", [16, 128, NT], BF16)
    h1_d = nc.dram_tensor("h1", [8, 128, NT], BF16)  # rows 0-3 fwd, 4-7 bwd
    h2_d = nc.dram_tensor("h2", [8, 128, NT], BF16)

    with tile.TileContext(nc) as tc:
        _emit(tc, locals(), T=T, V=V, NT=NT, NG=NG, NCH=NCH, W=W, NW=NW,
              CL=CL, NC2=NC2)
    nc.compile()
    return nc


def _emit(tc, d, *, T, V, NT, NG, NCH, W, NW, CL, NC2):
    import os
    PH = int(os.environ.get("BASS_PHASES", "6"))
    nc = tc.nc

    # persistent pool for things that live across phases
    with tc.tile_pool(name="persist", bufs=1) as perst, \
         tc.tile_pool(name="ps_persist", bufs=1, space="PSUM") as psper:

        ident = perst.tile([128, 128], F32)
        make_identity(nc, ident[:])

        # ==== phase 0: gather + transpose -> per-tile x.T (bf16) ====
        # x.T is split into NG tiles so the layer-1 GEMMs only depend on
        # the gather tiles of their own token window -> the PE starts the
        # GEMMs while later indirect-DMA gathers are still in flight.
        with tc.tile_pool(name="xtp", bufs=1) as xtp:
            if PH >= 1:
                xTs = [xtp.tile([128, 4, 128], BF16, name=f"xT{j}",
                                tag=f"xT{j}") for j in range(NG)]
                with tc.tile_pool(name="p0", bufs=4) as p0, \
                     tc.tile_pool(name="ps0", bufs=4, space="PSUM") as ps0:
                    gidx_sb = p0.tile([128, NG], I32, name="gidx")
                    nc.sync.dma_start(gidx_sb[:], d["gidx_in"][:])
                    for j in range(NG):
                        gx = p0.tile([128, E], F32, name="gx")
                        nc.gpsimd.indirect_dma_start(
                            out=gx[:], out_offset=None, in_=d["emb_in"][:],
                            in_offset=bass.IndirectOffsetOnAxis(
                                ap=gidx_sb[:, j:j + 1], axis=0))
                        for k in range(4):
                            pst = ps0.tile([128, 128], F32, name="ptr")
                            nc.tensor.transpose(pst[:],
                                                gx[:, 128 * k:128 * (k + 1)],
                                                ident[:])
                            nc.vector.tensor_copy(xTs[j][:, k, :], pst[:])

                # =========== phase 1: layer-1 input GEMMs ===========
                _ugemm(tc, d["wih1f_in"], d["b1f_in"], d["u1f_d"], 4, NT, W,
                       NW, rhs_tiles=xTs)
                _ugemm(tc, d["wih1b_in"], d["b1b_in"], d["u1b_d"], 4, NT, W,
                       NW, rhs_tiles=xTs)

        # =========== phase 2: layer-1 scans ===========
        if PH >= 2:
            _scan(tc, d["u1f_d"], d["u1b_d"], d["whh1f_in"], d["whh1b_in"],
                  d["h1_d"], NT, NCH)

        # =========== phase 3: layer-2 input GEMMs ===========
        if PH >= 3:
            _ugemm(tc, d["wih2f_in"], d["b2f_in"], d["u2f_d"], 8, NT, W, NW,
                   rhs_dram=d["h1_d"])
            _ugemm(tc, d["wih2b_in"], d["b2b_in"], d["u2b_d"], 8, NT, W, NW,
                   rhs_dram=d["h1_d"])

        # =========== phase 4: layer-2 scans ===========
        if PH >= 4:
            _scan(tc, d["u2f_d"], d["u2b_d"], d["whh2f_in"], d["whh2b_in"],
                  d["h2_d"], NT, NCH)

        # =========== phase 5: emissions ===========
        if PH < 5:
            with tc.tile_pool(name="stub", bufs=1) as stub:
                zo = stub.tile([1, 8], F32, name="zo")
                nc.vector.memset(zo[:], 0.0)
                nc.sync.dma_start(d["out_d"][:], zo[:])
            return
        em = perst.tile([L, NT], F32, name="em")
        with tc.tile_pool(name="pe", bufs=3) as pe, \
             tc.tile_pool(name="pse", bufs=4, space="PSUM") as pse:
            wo_st = pe.tile([128, 8, L], F32, name="wo_st")
            nc.sync.dma_start(
                wo_st[:], d["wout_in"][:].rearrange("(k p) l -> p k l", p=128))
            wo = pe.tile([128, 8, L], BF16, name="wo")
            nc.vector.tensor_copy(wo[:], wo_st[:])
            bout_sb = pe.tile([L, 1], F32, name="bout")
            nc.sync.dma_start(bout_sb[:], d["bout_in"][:])
            for w in range(NW):
                rh = pe.tile([128, 8, W], BF16, name="rh")
                nc.sync.dma_start(
                    rh[:],
                    d["h2_d"][:].rearrange("k p n -> p k n")[:, :,
                                                            ts(w, W)])
                pst = pse.tile([L, W], F32, name="pem")
                for k in range(8):
                    nc.tensor.matmul(pst[:], wo[:, k, :], rh[:, k, :],
                                     start=(k == 0), stop=(k == 7))
                nc.vector.tensor_add(
                    em[:, ts(w, W)], pst[:],
                    bout_sb[:, :].to_broadcast([L, W]))

        # =========== phase 6: CRF ===========
        if PH >= 6:
            _crf(tc, d, em, perst, psper, T, NT, CL, NC2)
        else:
            with tc.tile_pool(name="stub", bufs=1) as stub:
                zo = stub.tile([1, 8], F32, name="zo")
                nc.vector.memset(zo[:], 0.0)
                nc.sync.dma_start(d["out_d"][:], zo[:])


def _ugemm(tc, w_dram, b_dram, u_dram, KC, NT, W, NW, rhs_tiles=None,
           rhs_dram=None):
    """u.T[m-tile] = sum_k Wih.T[k,mtile].T @ rhs[k]  (+ bias), fp32r."""
    nc = tc.nc
    with tc.tile_pool(name="ug", bufs=2) as ug, \
         tc.tile_pool(name="ug_w", bufs=1) as ugw, \
         tc.tile_pool(name="ug_ps", bufs=4, space="PSUM") as ups:
        wt = ugw.tile([128, KC, 2048], BF16, name="wt")
        for k in range(KC):
            st = ug.tile([128, 2048], F32, name="wstage")
            nc.sync.dma_start(st[:], w_dram[ds(128 * k, 128), :])
            nc.vector.tensor_copy(wt[:, k, :], st[:])
        bias = ugw.tile([128, 16], F32, name="bias")
        nc.sync.dma_start(bias[:], b_dram[:])
        SUB = W // 128
        for w in range(NW):
            if rhs_tiles is None:
                rh_t = ug.tile([128, KC, W], BF16, name="rh_t")
                nc.sync.dma_start(
                    rh_t[:],
                    rhs_dram[:].rearrange("k p n -> p k n")[:, :, ts(w, W)])
                rh = rh_t[:]
            for m in range(16):
                pst = ups.tile([128, W], F32, name="pu")
                if rhs_tiles is not None:
                    for s in range(SUB):
                        for k in range(KC):
                            nc.tensor.matmul(
                                pst[:, 128 * s:128 * (s + 1)],
                                wt[:, k, 128 * m:128 * (m + 1)],
                                rhs_tiles[SUB * w + s][:, k, :],
                                start=(k == 0), stop=(k == KC - 1))
                else:
                    for k in range(KC):
                        nc.tensor.matmul(
                            pst[:], wt[:, k, 128 * m:128 * (m + 1)],
                            rh[:, k, :],
                            start=(k == 0), stop=(k == KC - 1))
                usb = ug.tile([128, W], BF16, name="usb")
                nc.vector.tensor_add(
                    usb[:], pst[:],
                    bias[:, m:m + 1].to_broadcast([128, W]))
                nc.sync.dma_start(u_dram[m, :, ts(w, W)], usb[:])


def _scan(tc, uf_dram, ub_dram, whf_dram, whb_dram, hT_dram, NT, NCH):
    """Interleaved fwd+bwd LSTM scans, bf16 recurrent matmuls."""
    nc = tc.nc
    with tc.tile_pool(name="sc_w", bufs=1) as scw, \
         tc.tile_pool(name="sc_st", bufs=1) as scs, \
         tc.tile_pool(name="sc", bufs=3) as sc, \
         tc.tile_pool(name="sc_ps", bufs=4, space="PSUM") as sps:
        wbf = []
        with tc.tile_pool(name="sc_wst", bufs=1) as scst:
            for wi, w_dram in enumerate((whf_dram, whb_dram)):
                st = scst.tile([128, 4, 2048], F32, name=f"whstage{wi}",
                               tag="whstage")
                nc.sync.dma_start(
                    st[:], w_dram[:].rearrange("(k p) m -> p k m", p=128))
                wb = scw.tile([128, 4, 2048], BF16, name=f"whbf{wi}",
                              tag=f"whbf{wi}")
                nc.vector.tensor_copy(wb[:], st[:])
                wbf.append(wb)

        hst = [scs.tile([128, 4, B], BF16, name=f"h_{i}") for i in range(2)]
        cst = [scs.tile([128, 4, B], F32, name=f"c_{i}") for i in range(2)]
        for t_ in hst + cst:
            nc.vector.memset(t_[:], 0.0)

        uf_ap = uf_dram[:].rearrange("m p n -> p m n")
        ub_ap = ub_dram[:].rearrange("m p n -> p m n")
        hT_f = hT_dram[ds(0, 4)].rearrange("k p n -> p k n")
        hT_b = hT_dram[ds(4, 4)].rearrange("k p n -> p k n")

        with tc.For_i(0, NCH, hint_engines=(mybir.EngineType.PE,),
                      name="scan") as ci:
            u_sb = []
            for di, u_ap in enumerate((uf_ap, ub_ap)):
                ut = sc.tile([128, 16, 128], BF16, name=f"u_{di}")
                if di == 0:
                    nc.sync.dma_start(ut[:], u_ap[:, :, ts(ci, 128)])
                else:
                    nc.sync.dma_start(
                        ut[:], u_ap[:, :, ds(ci * (-128) + (NT - 128), 128)])
                u_sb.append(ut)
            hout = [sc.tile([128, 4, 128], BF16, name=f"ho_{di}")
                    for di in range(2)]
            for j in range(16):
                for di in range(2):
                    h_bf, c_sb = hst[di], cst[di]
                    col = 8 * j if di == 0 else 8 * (15 - j)
                    pst = sps.tile([128, 128], F32, name=f"pg_{di}")
                    for m in range(16):
                        for k in range(4):
                            nc.tensor.matmul(
                                pst[:, 8 * m:8 * (m + 1)],
                                wbf[di][:, k, 128 * m:128 * (m + 1)],
                                h_bf[:, k, :],
                                start=(k == 0), stop=(k == 3))
                    g = sc.tile([128, 128], F32, name=f"g_{di}")
                    nc.vector.tensor_add(
                        g[:].rearrange("p (m b) -> p m b", m=16),
                        pst[:].rearrange("p (m b) -> p m b", m=16),
                        u_sb[di][:, :, ds(col, 8)])
                    sif = sc.tile([128, 64], F32, name=f"sif_{di}")
                    nc.scalar.activation(sif[:], g[:, 0:64], AF.Sigmoid)
                    tg = sc.tile([128, 32], F32, name=f"tg_{di}")
                    nc.scalar.activation(tg[:], g[:, 64:96], AF.Tanh)
                    so = sc.tile([128, 32], F32, name=f"so_{di}")
                    nc.scalar.activation(so[:], g[:, 96:128], AF.Sigmoid)
                    t1 = sc.tile([128, 32], F32, name=f"t1_{di}")
                    nc.vector.tensor_mul(
                        t1[:], sif[:, 32:64],
                        c_sb[:].rearrange("p k b -> p (k b)"))
                    t2 = sc.tile([128, 32], F32, name=f"t2_{di}")
                    nc.vector.tensor_mul(t2[:], sif[:, 0:32], tg[:])
                    nc.vector.tensor_add(
                        c_sb[:].rearrange("p k b -> p (k b)"), t1[:], t2[:])
                    tcn = sc.tile([128, 32], F32, name=f"tc_{di}")
                    nc.scalar.activation(
                        tcn[:], c_sb[:].rearrange("p k b -> p (k b)"),
                        AF.Tanh)
                    nc.vector.tensor_mul(
                        hout[di][:, :, ds(col, 8)],
                        so[:].rearrange("p (k b) -> p k b", k=4),
                        tcn[:].rearrange("p (k b) -> p k b", k=4))
                    nc.vector.tensor_mul(
                        h_bf[:].rearrange("p k b -> p (k b)"),
                        so[:], tcn[:])
            nc.sync.dma_start(hT_f[:, :, ts(ci, 128)], hout[0][:])
            nc.sync.dma_start(hT_b[:, :, ds(ci * (-128) + (NT - 128), 128)],
                              hout[1][:])


def _crf(tc, d, em, perst, psper, T, NT, CL, NC2):
    nc = tc.nc
    L_ = L
    with tc.tile_pool(name="cr", bufs=2) as cr, \
         tc.tile_pool(name="cr_st", bufs=1) as crs, \
         tc.tile_pool(name="cr_ps", bufs=2, space="PSUM") as cps, \
         tc.tile_pool(name="cr_ps1", bufs=1, space="PSUM") as cps1:
        # constants
        trans_sb = crs.tile([L_, L_], F32, name="trans")
        nc.sync.dma_start(trans_sb[:], d["trans_in"][:])
        P_sb = crs.tile([L_, L_], F32, name="P")
        nc.scalar.activation(P_sb[:], trans_sb[:], AF.Exp)
        start_sb = crs.tile([L_, 1], F32, name="start")
        nc.sync.dma_start(start_sb[:], d["start_in"][:])
        end_sb = crs.tile([L_, 1], F32, name="end")
        nc.sync.dma_start(end_sb[:], d["end_in"][:])
        id17 = crs.tile([L_, L_], F32, name="id17")
        make_identity(nc, id17[:])
        one17 = crs.tile([L_, 1], F32, name="one17")
        nc.vector.memset(one17[:], 1.0)
        onerow = crs.tile([1, L_], F32, name="onerow")
        nc.vector.memset(onerow[:], 1.0)
        moff_sb = crs.tile([L_, NT], U8, name="moff")
        nc.sync.dma_start(moff_sb[:], d["moff_in"][:])
        lenoff_sb = crs.tile([1, B], F32, name="lenoff")
        nc.sync.dma_start(lenoff_sb[:], d["lenoff_in"][:])
        out_sb = crs.tile([1, 8], F32, name="outv")
        nc.vector.memset(out_sb[:], 0.0)

        # Emask = exp(em - CREB), 1.0 where step inactive
        crebt = crs.tile([L_, 1], F32, name="crebt")
        nc.vector.memset(crebt[:], -CREB)
        emask = perst.tile([L_, NT], F32, name="emask")
        nc.scalar.activation(emask[:], em[:], AF.Exp, bias=crebt[:, :])
        nc.vector.copy_predicated(emask[:], moff_sb[:],
                                  one17[:, :].to_broadcast([L_, NT]))

        # ---- blocked exp-domain chunk scan ----
        # state At[k, (c, b, i)] = (prod_{tau} P diag(E))^T per chunk c
        NFREE = CC * B * L_
        A = [crs.tile([L_, CC, B, L_], F32, name=f"A{i}") for i in range(2)]
        nc.vector.tensor_copy(
            A[0][:],
            id17[:, None, None, :].to_broadcast([L_, CC, B, L_]))
        NGRP = 4
        CG = CC // NGRP          # chunks per matmul group
        GW = CG * B * L_         # 272
        for tau in range(CL):
            src, dst = A[tau % 2], A[(tau + 1) % 2]
            emsl = emask[:].rearrange("p (c s b) -> p c s b", c=CC,
                                      s=CL)[:, :, tau, :]
            mosl = moff_sb[:].rearrange("p (c s b) -> p c s b", c=CC,
                                        s=CL)[:, :, tau, :]
            for gp in range(NGRP):
                cs = slice(CG * gp, CG * (gp + 1))
                pA = cps.tile([L_, GW], F32, name="pA")
                nc.tensor.matmul(
                    pA[:], P_sb[:],
                    src[:].rearrange("p c b i -> p (c b i)")[:, ts(gp, GW)],
                    start=True, stop=True)
                nc.vector.tensor_mul(
                    dst[:, cs, :, :],
                    pA[:].rearrange("p (c b i) -> p c b i", c=CG, b=B),
                    emsl[:, cs, :, None].to_broadcast([L_, CG, B, L_]))
            nc.vector.copy_predicated(
                dst[:],
                mosl[:, :, :, None].to_broadcast([L_, CC, B, L_]),
                src[:])
        Afin = A[CL % 2]
        logA = perst.tile([L_, CC, B, L_], F32, name="logA")
        nc.scalar.activation(logA[:], Afin[:], AF.Ln)
        nc.vector.tensor_scalar_max(logA[:], logA[:], -1e30)

        # ---- log-domain combine ----
        s_fm = crs.tile([L_, B], F32, name="s_fm")   # s[j, b]
        nc.vector.tensor_add(s_fm[:], em[:, 0:B],
                             start_sb[:, :].to_broadcast([L_, B]))
        s_rep = crs.tile([L_, B, L_], F32, name="s_rep")

        def replicate(src_fm):
            # s_rep[k, (b, i)] = src_fm[i, b] for all k
            pT = cps1.tile([B, L_], F32, name="pT")
            nc.tensor.transpose(pT[:], src_fm[:], id17[:])
            sT = cr.tile([B, L_], F32, name="sT")
            nc.vector.tensor_copy(sT[:], pT[:])
            srow = cr.tile([1, B * L_], F32, name="srow")
            nc.sync.dma_start(srow[:], sT[:])
            pR = cps1.tile([L_, B * L_], F32, name="pR")
            nc.tensor.matmul(pR[:], onerow[:], srow[:], start=True, stop=True)
            nc.vector.tensor_copy(
                s_rep[:], pR[:].rearrange("p (b i) -> p b i", b=B))

        replicate(s_fm)
        for c in range(CC):
            tmp = cr.tile([L_, B, L_], F32, name="ctmp")
            nc.vector.tensor_add(tmp[:], logA[:, c, :, :], s_rep[:])
            etmp = cr.tile([L_, B, L_], F32, name="cetmp")
            nc.scalar.activation(etmp[:], tmp[:], AF.Exp)
            sex = cr.tile([L_, B], F32, name="sex")
            nc.vector.tensor_reduce(sex[:], etmp[:], AX.X, OP.add)
            nc.scalar.activation(s_fm[:], sex[:], AF.Ln)
            if c < CC - 1:
                replicate(s_fm)

        # logZ_b = LSE_k(s[k,b] + end[k]) + lenoff[b]; out slot0 = sum_b
        send = cr.tile([L_, B], F32, name="send")
        nc.vector.tensor_add(send[:], s_fm[:],
                             end_sb[:, :].to_broadcast([L_, B]))
        eend = cr.tile([L_, B], F32, name="eend")
        nc.scalar.activation(eend[:], send[:], AF.Exp)
        pz = cps1.tile([1, B], F32, name="pz")
        nc.tensor.matmul(pz[:], one17[:], eend[:], start=True, stop=True)
        lz = cr.tile([1, B], F32, name="lz")
        nc.scalar.activation(lz[:], pz[:], AF.Ln)
        nc.vector.tensor_add(lz[:], lz[:], lenoff_sb[:])
        nc.vector.tensor_reduce(out_sb[:, 0:1], lz[:], AX.X, OP.add)

        # ---- numerator parts ----
        def dot_to_slot(vec_lp, slot):
            # vec_lp: [L, 1] -> sum over partitions into out_sb[0, slot]
            pd = cps1.tile([1, 1], F32, name="pd")
            nc.tensor.matmul(pd[:], one17[:], vec_lp[:], start=True,
                             stop=True)
            nc.vector.tensor_copy(out_sb[:, slot:slot + 1], pd[:])

        # e_tag part
        wm_sb = cr.tile([L_, NT], F32, name="wm")
        nc.sync.dma_start(wm_sb[:], d["wmask_in"][:])
        prod = cr.tile([L_, NT], F32, name="prod")
        nc.vector.tensor_mul(prod[:], em[:], wm_sb[:])
        r1 = cr.tile([L_, 1], F32, name="r1")
        nc.vector.tensor_reduce(r1[:], prod[:], AX.X, OP.add)
        dot_to_slot(r1, 1)

        # trans part: C = ohprevM.T-ish contraction, then <C, trans>
        ohp = cr.tile([128, NC2, L_], F32, name="ohp")
        nc.sync.dma_start(ohp[:],
                          d["ohprev_in"][:].rearrange("(c p) l -> p c l",
                                                      p=128))
        ohn = cr.tile([128, NC2, L_], F32, name="ohn")
        nc.sync.dma_start(ohn[:],
                          d["ohnext_in"][:].rearrange("(c p) l -> p c l",
                                                      p=128))
        pC = cps1.tile([L_, L_], F32, name="pC")
        for c2 in range(NC2):
            nc.tensor.matmul(pC[:], ohp[:, c2, :], ohn[:, c2, :],
                             start=(c2 == 0), stop=(c2 == NC2 - 1))
        tC = cr.tile([L_, L_], F32, name="tC")
        nc.vector.tensor_mul(tC[:], pC[:], trans_sb[:])
        r2 = cr.tile([L_, 1], F32, name="r2")
        nc.vector.tensor_reduce(r2[:], tC[:], AX.X, OP.add)
        dot_to_slot(r2, 2)

        # start / end parts
        oh0_sb = cr.tile([L_, B], F32, name="oh0")
        nc.sync.dma_start(oh0_sb[:], d["oh0_in"][:])
        t0 = cr.tile([L_, B], F32, name="t0")
        nc.vector.tensor_mul(t0[:], oh0_sb[:],
                             start_sb[:, :].to_broadcast([L_, B]))
        r3 = cr.tile([L_, 1], F32, name="r3")
        nc.vector.tensor_reduce(r3[:], t0[:], AX.X, OP.add)
        dot_to_slot(r3, 3)

        ohl_sb = cr.tile([L_, B], F32, name="ohl")
        nc.sync.dma_start(ohl_sb[:], d["ohlast_in"][:])
        t4 = cr.tile([L_, B], F32, name="t4")
        nc.vector.tensor_mul(t4[:], ohl_sb[:],
                             end_sb[:, :].to_broadcast([L_, B]))
        r4 = cr.tile([L_, 1], F32, name="r4")
        nc.vector.tensor_reduce(r4[:], t4[:], AX.X, OP.add)
        dot_to_slot(r4, 4)

        nc.sync.dma_start(d["out_d"][:], out_sb[:])


def _prep_core(core, sentences, mask, labels, T):
    """Per-core numpy input prep (index/layout only)."""
    NT = T * B
    NG = NT // 128
    bs = slice(B * core, B * (core + 1))
    sent = np.asarray(sentences[bs], dtype=np.int64)
    msk = np.asarray(mask[bs], dtype=bool)
    lab = np.asarray(labels[bs], dtype=np.int64)
    lens = msk.sum(axis=1).astype(np.int64)

    cols = np.arange(NT)
    tt, bb = cols // B, cols % B
    gidx = sent[bb, tt].astype(np.int32).reshape(NG, 128).T.copy()

    maskf = msk.astype(np.float32)
    lab_t = lab[bb, tt]                      # [NT]
    wmask = np.zeros((L, NT), np.float32)
    wmask[lab_t, cols] = maskf[bb, tt]

    ohprev = np.zeros((NT, L), np.float32)
    ohnext = np.zeros((NT, L), np.float32)
    valid_prev = tt >= 1
    lab_prev = lab[bb[valid_prev], tt[valid_prev] - 1]
    ohprev[cols[valid_prev], lab_prev] = maskf[bb[valid_prev],
                                               tt[valid_prev]]
    ohnext[cols, lab_t] = 1.0

    oh0 = np.zeros((L, B), np.float32)
    oh0[lab[:, 0], np.arange(B)] = 1.0
    ohlast = np.zeros((L, B), np.float32)
    ohlast[lab[np.arange(B), lens - 1], np.arange(B)] = 1.0

    lenoff = ((lens - 1).astype(np.float32) * CREB)[None, :]

    inactive = (tt == 0) | (tt >= lens[bb])
    moff = np.broadcast_to(inactive[None, :], (L, NT)).astype(np.uint8).copy()

    return {"gidx": gidx, "wmask": wmask, "ohprev": ohprev,
            "ohnext": ohnext, "oh0": oh0, "ohlast": ohlast,
            "lenoff": lenoff.astype(np.float32), "moff": moff}


def _prep_shared(emb, lstm_params, W_out, b_out, start_t, end_t, trans):
    def f32c(x):
        return np.ascontiguousarray(np.asarray(x), dtype=np.float32)

    (w1f, wh1f, bf1, w1b, wh1b, bb1), (w2f, wh2f, bf2, w2b, wh2b, bb2) = \
        lstm_params

    def bias_fm(b_):
        return np.ascontiguousarray(f32c(b_).reshape(16, 128).T)

    return {
        "emb": f32c(emb),
        "wih1f": np.ascontiguousarray(f32c(w1f).T),
        "wih1b": np.ascontiguousarray(f32c(w1b).T),
        "whh1f": np.ascontiguousarray(f32c(wh1f).T),
        "whh1b": np.ascontiguousarray(f32c(wh1b).T),
        "wih2f": np.ascontiguousarray(f32c(w2f).T),
        "wih2b": np.ascontiguousarray(f32c(w2b).T),
        "whh2f": np.ascontiguousarray(f32c(wh2f).T),
        "whh2b": np.ascontiguousarray(f32c(wh2b).T),
        "b1f": bias_fm(bf1), "b1b": bias_fm(bb1),
        "b2f": bias_fm(bf2), "b2b": bias_fm(bb2),
        "wout": np.ascontiguousarray(f32c(W_out).T),
        "bout": f32c(b_out).reshape(L, 1),
        "trans": f32c(trans),
        "start": f32c(start_t).reshape(L, 1),
        "end": f32c(end_t).reshape(L, 1),
    }


def run(sentences, mask, labels, emb, lstm_params, W_out, b_out, start_t,
        end_t, trans, T=None, V=None):
    T = T if T is not None else np.asarray(sentences).shape[1]
    V = V if V is not None else np.asarray(emb).shape[0]
    import os
    key = (T, V, os.environ.get("BASS_PHASES", "6"))
    if key not in _CACHE:
        _CACHE[key] = _build(T, V)
    nc = _CACHE[key]

    shared = _prep_shared(emb, lstm_params, W_out, b_out, start_t, end_t,
                          trans)
    in_maps = []
    for core in range(8):
        m = dict(shared)
        m.update(_prep_core(core, sentences, mask, labels, T))
        in_maps.append(m)
    res = run_bass_kernel_spmd(nc, in_maps, list(range(8)))
    total = 0.0
    for core in range(8):
        o = res.results[core]["out"][0]
        total += float(o[0]) - float(o[1] + o[2] + o[3] + o[4])
    return np.float32(total / 64.0)


def kernel(sentences, mask, labels, emb, lstm_params, W_out, b_out, start_t,
           end_t, trans):
    return run(sentences, mask, labels, emb, lstm_params, W_out, b_out,
               start_t, end_t, trans)


# revision 22
# speedup vs baseline: 1.0733x; 1.0388x over previous
"""BiLSTM-CRF forward loss on 8 Trainium2 NeuronCores.

Strategy: pure data-parallel over batch (8 sequences per core, no
cross-core communication).  Per core:
  1. embedding gather (indirect DMA) + PE-transpose -> x.T feature-major
  2. input-projection GEMMs (fp32r, full rate at N=512) -> u = x @ Wih.T + b
  3. two BiLSTM layers: forward+backward scans interleaved on the same
     core; recurrent matmuls in bf16 with weights stationary
     (feature-major h, no transposes in the loop)
  4. emission GEMM (fp32r) -> emissions feature-major [17, T*8]
  5. CRF partition function via a blocked exp-domain scan (8 parallel
     chunk products, log-domain combine) + numerator via masked one-hot
     contractions.  Each core returns 5 scalars; the host combines them.
"""

import sys

for _p in ("/opt/trn_rl_repo", "/root/.axon_site/_ro/trn_rl_repo"):
    if _p not in sys.path:
        sys.path.insert(0, _p)

import numpy as np

import concourse.bass as bass
import concourse.bacc as bacc
import concourse.mybir as mybir
import concourse.tile as tile
from concourse.bass import ds, ts
from concourse.bass_utils import run_bass_kernel_spmd
from concourse.masks import make_identity

F32 = mybir.dt.float32
F32R = mybir.dt.float32r
BF16 = mybir.dt.bfloat16
I32 = mybir.dt.int32
U8 = mybir.dt.uint8
AF = mybir.ActivationFunctionType
OP = mybir.AluOpType
AX = mybir.AxisListType

B = 8            # sequences per core
E = 512          # embedding dim
H = 512          # hidden per direction
G = 2048         # 4*H gate rows
L = 17           # number of tags
CREB = 2.83      # CRF exp-domain rebase constant (log-domain growth/step)
CC = 8           # CRF chunk count

_CACHE = {}


def _build(T, V):
    NT = T * B                 # tokens per core, time-major cols (t*B + b)
    NG = NT // 128             # gather tiles of 128 tokens
    NCH = T // 32              # scan chunks of 32 timesteps
    W = min(512, NT)           # GEMM window (moving free dim)
    NW = NT // W
    CL = T // CC               # CRF chunk length
    NC2 = NT // 128            # 128-col chunks for the trans-count matmul

    nc = bacc.Bacc(None, target_bir_lowering=False, debug=False,
                   num_swdge_queues=4)

    # ---------------- kernel I/O ----------------
    emb_in = nc.dram_tensor("emb", [V, E], F32, kind="ExternalInput")
    gidx_in = nc.dram_tensor("gidx", [128, NG], I32, kind="ExternalInput")
    wih1f_in = nc.dram_tensor("wih1f", [E, G], F32, kind="ExternalInput")
    wih1b_in = nc.dram_tensor("wih1b", [E, G], F32, kind="ExternalInput")
    whh1f_in = nc.dram_tensor("whh1f", [H, G], F32, kind="ExternalInput")
    whh1b_in = nc.dram_tensor("whh1b", [H, G], F32, kind="ExternalInput")
    wih2f_in = nc.dram_tensor("wih2f", [2 * H, G], F32, kind="ExternalInput")
    wih2b_in = nc.dram_tensor("wih2b", [2 * H, G], F32, kind="ExternalInput")
    whh2f_in = nc.dram_tensor("whh2f", [H, G], F32, kind="ExternalInput")
    whh2b_in = nc.dram_tensor("whh2b", [H, G], F32, kind="ExternalInput")
    b1f_in = nc.dram_tensor("b1f", [128, 16], F32, kind="ExternalInput")
    b1b_in = nc.dram_tensor("b1b", [128, 16], F32, kind="ExternalInput")
    b2f_in = nc.dram_tensor("b2f", [128, 16], F32, kind="ExternalInput")
    b2b_in = nc.dram_tensor("b2b", [128, 16], F32, kind="ExternalInput")
    wout_in = nc.dram_tensor("wout", [2 * H, L], F32, kind="ExternalInput")
    bout_in = nc.dram_tensor("bout", [L, 1], F32, kind="ExternalInput")
    trans_in = nc.dram_tensor("trans", [L, L], F32, kind="ExternalInput")
    start_in = nc.dram_tensor("start", [L, 1], F32, kind="ExternalInput")
    end_in = nc.dram_tensor("end", [L, 1], F32, kind="ExternalInput")
    wmask_in = nc.dram_tensor("wmask", [L, NT], F32, kind="ExternalInput")
    ohprev_in = nc.dram_tensor("ohprev", [NT, L], F32, kind="ExternalInput")
    ohnext_in = nc.dram_tensor("ohnext", [NT, L], F32, kind="ExternalInput")
    oh0_in = nc.dram_tensor("oh0", [L, B], F32, kind="ExternalInput")
    ohlast_in = nc.dram_tensor("ohlast", [L, B], F32, kind="ExternalInput")
    lenoff_in = nc.dram_tensor("lenoff", [1, B], F32, kind="ExternalInput")
    moff_in = nc.dram_tensor("moff", [L, NT], U8, kind="ExternalInput")

    out_d = nc.dram_tensor("out", [1, 8], F32, kind="ExternalOutput")

    # ---------------- internal DRAM ----------------
    u1f_d = nc.dram_tensor("u1f", [16, 128, NT], BF16)
    u1b_d = nc.dram_tensor("u1b", [16, 128, NT], BF16)
    u2f_d = nc.dram_tensor("u2f", [16, 128, NT], BF16)
    u2b_d = nc.dram_tensor("u2b", [16, 128, NT], BF16)
    h1_d = nc.dram_tensor("h1", [8, 128, NT], BF16)  # rows 0-3 fwd, 4-7 bwd
    h2_d = nc.dram_tensor("h2", [8, 128, NT], BF16)

    with tile.TileContext(nc) as tc:
        _emit(tc, locals(), T=T, V=V, NT=NT, NG=NG, NCH=NCH, W=W, NW=NW,
              CL=CL, NC2=NC2)
    nc.compile()
    return nc


def _emit(tc, d, *, T, V, NT, NG, NCH, W, NW, CL, NC2):
    import os
    PH = int(os.environ.get("BASS_PHASES", "6"))
    nc = tc.nc

    # persistent pool for things that live across phases
    with tc.tile_pool(name="persist", bufs=1) as perst, \
         tc.tile_pool(name="ps_persist", bufs=1, space="PSUM") as psper:

        ident = perst.tile([128, 128], F32)
        make_identity(nc, ident[:])

        # ==== phase 0: gather + transpose -> per-tile x.T (bf16) ====
        # x.T is split into NG tiles so the layer-1 GEMMs only depend on
        # the gather tiles of their own token window -> the PE starts the
        # GEMMs while later indirect-DMA gathers are still in flight.
        with tc.tile_pool(name="xtp", bufs=1) as xtp:
            if PH >= 1:
                xTs = [xtp.tile([128, 4, 128], BF16, name=f"xT{j}",
                                tag=f"xT{j}") for j in range(NG)]
                with tc.tile_pool(name="p0", bufs=4) as p0, \
                     tc.tile_pool(name="ps0", bufs=4, space="PSUM") as ps0:
                    gidx_sb = p0.tile([128, NG], I32, name="gidx")
                    nc.sync.dma_start(gidx_sb[:], d["gidx_in"][:])
                    for j in range(NG):
                        gx = p0.tile([128, E], F32, name="gx")
                        nc.gpsimd.indirect_dma_start(
                            out=gx[:], out_offset=None, in_=d["emb_in"][:],
                            in_offset=bass.IndirectOffsetOnAxis(
                                ap=gidx_sb[:, j:j + 1], axis=0))
                        for k in range(4):
                            pst = ps0.tile([128, 128], F32, name="ptr")
                            nc.tensor.transpose(pst[:],
                                                gx[:, 128 * k:128 * (k + 1)],
                                                ident[:])
                            nc.vector.tensor_copy(xTs[j][:, k, :], pst[:])

                # =========== phase 1: layer-1 input GEMMs ===========
                _ugemm(tc, d["wih1f_in"], d["b1f_in"], d["u1f_d"], 4, NT, W,
                       NW, rhs_tiles=xTs)
                _ugemm(tc, d["wih1b_in"], d["b1b_in"], d["u1b_d"], 4, NT, W,
                       NW, rhs_tiles=xTs)

        # =========== phase 2: layer-1 scans ===========
        if PH >= 2:
            _scan(tc, d["u1f_d"], d["u1b_d"], d["whh1f_in"], d["whh1b_in"],
                  d["h1_d"], NT, NCH)

        # =========== phase 3: layer-2 input GEMMs ===========
        if PH >= 3:
            _ugemm(tc, d["wih2f_in"], d["b2f_in"], d["u2f_d"], 8, NT, W, NW,
                   rhs_dram=d["h1_d"])
            _ugemm(tc, d["wih2b_in"], d["b2b_in"], d["u2b_d"], 8, NT, W, NW,
                   rhs_dram=d["h1_d"])

        # =========== phase 4: layer-2 scans ===========
        if PH >= 4:
            _scan(tc, d["u2f_d"], d["u2b_d"], d["whh2f_in"], d["whh2b_in"],
                  d["h2_d"], NT, NCH)

        # =========== phase 5: emissions ===========
        if PH < 5:
            with tc.tile_pool(name="stub", bufs=1) as stub:
                zo = stub.tile([1, 8], F32, name="zo")
                nc.vector.memset(zo[:], 0.0)
                nc.sync.dma_start(d["out_d"][:], zo[:])
            return
        em = perst.tile([L, NT], F32, name="em")
        with tc.tile_pool(name="pe", bufs=3) as pe, \
             tc.tile_pool(name="pse", bufs=4, space="PSUM") as pse:
            wo_st = pe.tile([128, 8, L], F32, name="wo_st")
            nc.sync.dma_start(
                wo_st[:], d["wout_in"][:].rearrange("(k p) l -> p k l", p=128))
            wo = pe.tile([128, 8, L], BF16, name="wo")
            nc.vector.tensor_copy(wo[:], wo_st[:])
            bout_sb = pe.tile([L, 1], F32, name="bout")
            nc.sync.dma_start(bout_sb[:], d["bout_in"][:])
            for w in range(NW):
                rh = pe.tile([128, 8, W], BF16, name="rh")
                nc.sync.dma_start(
                    rh[:],
                    d["h2_d"][:].rearrange("k p n -> p k n")[:, :,
                                                            ts(w, W)])
                pst = pse.tile([L, W], F32, name="pem")
                for k in range(8):
                    nc.tensor.matmul(pst[:], wo[:, k, :], rh[:, k, :],
                                     start=(k == 0), stop=(k == 7))
                nc.vector.tensor_add(
                    em[:, ts(w, W)], pst[:],
                    bout_sb[:, :].to_broadcast([L, W]))

        # =========== phase 6: CRF ===========
        if PH >= 6:
            _crf(tc, d, em, perst, psper, T, NT, CL, NC2)
        else:
            with tc.tile_pool(name="stub", bufs=1) as stub:
                zo = stub.tile([1, 8], F32, name="zo")
                nc.vector.memset(zo[:], 0.0)
                nc.sync.dma_start(d["out_d"][:], zo[:])


def _ugemm(tc, w_dram, b_dram, u_dram, KC, NT, W, NW, rhs_tiles=None,
           rhs_dram=None):
    """u.T[m-tile] = sum_k Wih.T[k,mtile].T @ rhs[k]  (+ bias), fp32r."""
    nc = tc.nc
    with tc.tile_pool(name="ug", bufs=2) as ug, \
         tc.tile_pool(name="ug_w", bufs=1) as ugw, \
         tc.tile_pool(name="ug_ps", bufs=4, space="PSUM") as ups:
        wt = ugw.tile([128, KC, 2048], BF16, name="wt")
        for k in range(KC):
            st = ug.tile([128, 2048], F32, name="wstage")
            nc.sync.dma_start(st[:], w_dram[ds(128 * k, 128), :])
            nc.vector.tensor_copy(wt[:, k, :], st[:])
        bias = ugw.tile([128, 16], F32, name="bias")
        nc.sync.dma_start(bias[:], b_dram[:])
        SUB = W // 128
        for w in range(NW):
            if rhs_tiles is None:
                rh_t = ug.tile([128, KC, W], BF16, name="rh_t")
                nc.sync.dma_start(
                    rh_t[:],
                    rhs_dram[:].rearrange("k p n -> p k n")[:, :, ts(w, W)])
                rh = rh_t[:]
            for m in range(16):
                pst = ups.tile([128, W], F32, name="pu")
                if rhs_tiles is not None:
                    for s in range(SUB):
                        for k in range(KC):
                            nc.tensor.matmul(
                                pst[:, 128 * s:128 * (s + 1)],
                                wt[:, k, 128 * m:128 * (m + 1)],
                                rhs_tiles[SUB * w + s][:, k, :],
                                start=(k == 0), stop=(k == KC - 1))
                else:
                    for k in range(KC):
                        nc.tensor.matmul(
                            pst[:], wt[:, k, 128 * m:128 * (m + 1)],
                            rh[:, k, :],
                            start=(k == 0), stop=(k == KC - 1))
                usb = ug.tile([128, W], BF16, name="usb")
                nc.vector.tensor_add(
                    usb[:], pst[:],
                    bias[:, m:m + 1].to_broadcast([128, W]))
                nc.sync.dma_start(u_dram[m, :, ts(w, W)], usb[:])


def _scan(tc, uf_dram, ub_dram, whf_dram, whb_dram, hT_dram, NT, NCH):
    """Interleaved fwd+bwd LSTM scans, bf16 recurrent matmuls."""
    nc = tc.nc
    with tc.tile_pool(name="sc_w", bufs=1) as scw, \
         tc.tile_pool(name="sc_st", bufs=1) as scs, \
         tc.tile_pool(name="sc", bufs=3) as sc, \
         tc.tile_pool(name="sc_ps", bufs=4, space="PSUM") as sps:
        wbf = []
        with tc.tile_pool(name="sc_wst", bufs=1) as scst:
            for wi, w_dram in enumerate((whf_dram, whb_dram)):
                st = scst.tile([128, 4, 2048], F32, name=f"whstage{wi}",
                               tag="whstage")
                nc.sync.dma_start(
                    st[:], w_dram[:].rearrange("(k p) m -> p k m", p=128))
                wb = scw.tile([128, 4, 2048], BF16, name=f"whbf{wi}",
                              tag=f"whbf{wi}")
                nc.vector.tensor_copy(wb[:], st[:])
                wbf.append(wb)

        hst = [scs.tile([128, 4, B], BF16, name=f"h_{i}") for i in range(2)]
        cst = [scs.tile([128, 4, B], F32, name=f"c_{i}") for i in range(2)]
        for t_ in hst + cst:
            nc.vector.memset(t_[:], 0.0)

        uf_ap = uf_dram[:].rearrange("m p n -> p m n")
        ub_ap = ub_dram[:].rearrange("m p n -> p m n")
        hT_f = hT_dram[ds(0, 4)].rearrange("k p n -> p k n")
        hT_b = hT_dram[ds(4, 4)].rearrange("k p n -> p k n")

        with tc.For_i(0, NCH, hint_engines=(mybir.EngineType.PE,),
                      name="scan") as ci:
            u_sb = []
            for di, u_ap in enumerate((uf_ap, ub_ap)):
                ut = sc.tile([128, 16, 256], BF16, name=f"u_{di}")
                if di == 0:
                    nc.sync.dma_start(ut[:], u_ap[:, :, ts(ci, 256)])
                else:
                    nc.sync.dma_start(
                        ut[:], u_ap[:, :, ds(ci * (-256) + (NT - 256), 256)])
                u_sb.append(ut)
            hout = [sc.tile([128, 4, 256], BF16, name=f"ho_{di}")
                    for di in range(2)]
            for j in range(32):
                for di in range(2):
                    h_bf, c_sb = hst[di], cst[di]
                    col = 8 * j if di == 0 else 8 * (31 - j)
                    pst = sps.tile([128, 128], F32, name=f"pg_{di}")
                    for m in range(16):
                        for k in range(4):
                            nc.tensor.matmul(
                                pst[:, 8 * m:8 * (m + 1)],
                                wbf[di][:, k, 128 * m:128 * (m + 1)],
                                h_bf[:, k, :],
                                start=(k == 0), stop=(k == 3))
                    g = sc.tile([128, 128], F32, name=f"g_{di}")
                    nc.vector.tensor_add(
                        g[:].rearrange("p (m b) -> p m b", m=16),
                        pst[:].rearrange("p (m b) -> p m b", m=16),
                        u_sb[di][:, :, ds(col, 8)])
                    sif = sc.tile([128, 64], F32, name=f"sif_{di}")
                    nc.scalar.activation(sif[:], g[:, 0:64], AF.Sigmoid)
                    tg = sc.tile([128, 32], F32, name=f"tg_{di}")
                    nc.scalar.activation(tg[:], g[:, 64:96], AF.Tanh)
                    so = sc.tile([128, 32], F32, name=f"so_{di}")
                    nc.scalar.activation(so[:], g[:, 96:128], AF.Sigmoid)
                    t1 = sc.tile([128, 32], F32, name=f"t1_{di}")
                    nc.vector.tensor_mul(
                        t1[:], sif[:, 32:64],
                        c_sb[:].rearrange("p k b -> p (k b)"))
                    t2 = sc.tile([128, 32], F32, name=f"t2_{di}")
                    nc.vector.tensor_mul(t2[:], sif[:, 0:32], tg[:])
                    nc.vector.tensor_add(
                        c_sb[:].rearrange("p k b -> p (k b)"), t1[:], t2[:])
                    tcn = sc.tile([128, 32], F32, name=f"tc_{di}")
                    nc.scalar.activation(
                        tcn[:], c_sb[:].rearrange("p k b -> p (k b)"),
                        AF.Tanh)
                    nc.vector.tensor_mul(
                        hout[di][:, :, ds(col, 8)],
                        so[:].rearrange("p (k b) -> p k b", k=4),
                        tcn[:].rearrange("p (k b) -> p k b", k=4))
                    nc.vector.tensor_mul(
                        h_bf[:].rearrange("p k b -> p (k b)"),
                        so[:], tcn[:])
            nc.sync.dma_start(hT_f[:, :, ts(ci, 256)], hout[0][:])
            nc.sync.dma_start(hT_b[:, :, ds(ci * (-256) + (NT - 256), 256)],
                              hout[1][:])


def _crf(tc, d, em, perst, psper, T, NT, CL, NC2):
    nc = tc.nc
    L_ = L
    with tc.tile_pool(name="cr", bufs=2) as cr, \
         tc.tile_pool(name="cr_st", bufs=1) as crs, \
         tc.tile_pool(name="cr_ps", bufs=2, space="PSUM") as cps, \
         tc.tile_pool(name="cr_ps1", bufs=1, space="PSUM") as cps1:
        # constants
        trans_sb = crs.tile([L_, L_], F32, name="trans")
        nc.sync.dma_start(trans_sb[:], d["trans_in"][:])
        P_sb = crs.tile([L_, L_], F32, name="P")
        nc.scalar.activation(P_sb[:], trans_sb[:], AF.Exp)
        start_sb = crs.tile([L_, 1], F32, name="start")
        nc.sync.dma_start(start_sb[:], d["start_in"][:])
        end_sb = crs.tile([L_, 1], F32, name="end")
        nc.sync.dma_start(end_sb[:], d["end_in"][:])
        id17 = crs.tile([L_, L_], F32, name="id17")
        make_identity(nc, id17[:])
        one17 = crs.tile([L_, 1], F32, name="one17")
        nc.vector.memset(one17[:], 1.0)
        onerow = crs.tile([1, L_], F32, name="onerow")
        nc.vector.memset(onerow[:], 1.0)
        moff_sb = crs.tile([L_, NT], U8, name="moff")
        nc.sync.dma_start(moff_sb[:], d["moff_in"][:])
        lenoff_sb = crs.tile([1, B], F32, name="lenoff")
        nc.sync.dma_start(lenoff_sb[:], d["lenoff_in"][:])
        out_sb = crs.tile([1, 8], F32, name="outv")
        nc.vector.memset(out_sb[:], 0.0)

        # Emask = exp(em - CREB), 1.0 where step inactive
        crebt = crs.tile([L_, 1], F32, name="crebt")
        nc.vector.memset(crebt[:], -CREB)
        emask = perst.tile([L_, NT], F32, name="emask")
        nc.scalar.activation(emask[:], em[:], AF.Exp, bias=crebt[:, :])
        nc.vector.copy_predicated(emask[:], moff_sb[:],
                                  one17[:, :].to_broadcast([L_, NT]))

        # ---- blocked exp-domain chunk scan ----
        # state At[k, (c, b, i)] = (prod_{tau} P diag(E))^T per chunk c
        NFREE = CC * B * L_
        A = [crs.tile([L_, CC, B, L_], F32, name=f"A{i}") for i in range(2)]
        nc.vector.tensor_copy(
            A[0][:],
            id17[:, None, None, :].to_broadcast([L_, CC, B, L_]))
        NGRP = 4
        CG = CC // NGRP          # chunks per matmul group
        GW = CG * B * L_         # 272
        for tau in range(CL):
            src, dst = A[tau % 2], A[(tau + 1) % 2]
            emsl = emask[:].rearrange("p (c s b) -> p c s b", c=CC,
                                      s=CL)[:, :, tau, :]
            mosl = moff_sb[:].rearrange("p (c s b) -> p c s b", c=CC,
                                        s=CL)[:, :, tau, :]
            for gp in range(NGRP):
                cs = slice(CG * gp, CG * (gp + 1))
                pA = cps.tile([L_, GW], F32, name="pA")
                nc.tensor.matmul(
                    pA[:], P_sb[:],
                    src[:].rearrange("p c b i -> p (c b i)")[:, ts(gp, GW)],
                    start=True, stop=True)
                nc.vector.tensor_mul(
                    dst[:, cs, :, :],
                    pA[:].rearrange("p (c b i) -> p c b i", c=CG, b=B),
                    emsl[:, cs, :, None].to_broadcast([L_, CG, B, L_]))
            nc.vector.copy_predicated(
                dst[:],
                mosl[:, :, :, None].to_broadcast([L_, CC, B, L_]),
                src[:])
        Afin = A[CL % 2]
        logA = perst.tile([L_, CC, B, L_], F32, name="logA")
        nc.scalar.activation(logA[:], Afin[:], AF.Ln)
        nc.vector.tensor_scalar_max(logA[:], logA[:], -1e30)

        # ---- log-domain combine ----
        s_fm = crs.tile([L_, B], F32, name="s_fm")   # s[j, b]
        nc.vector.tensor_add(s_fm[:], em[:, 0:B],
                             start_sb[:, :].to_broadcast([L_, B]))
        s_rep = crs.tile([L_, B, L_], F32, name="s_rep")

        def replicate(src_fm):
            # s_rep[k, (b, i)] = src_fm[i, b] for all k
            pT = cps1.tile([B, L_], F32, name="pT")
            nc.tensor.transpose(pT[:], src_fm[:], id17[:])
            sT = cr.tile([B, L_], F32, name="sT")
            nc.vector.tensor_copy(sT[:], pT[:])
            srow = cr.tile([1, B * L_], F32, name="srow")
            nc.sync.dma_start(srow[:], sT[:])
            pR = cps1.tile([L_, B * L_], F32, name="pR")
            nc.tensor.matmul(pR[:], onerow[:], srow[:], start=True, stop=True)
            nc.vector.tensor_copy(
                s_rep[:], pR[:].rearrange("p (b i) -> p b i", b=B))

        replicate(s_fm)
        for c in range(CC):
            tmp = cr.tile([L_, B, L_], F32, name="ctmp")
            nc.vector.tensor_add(tmp[:], logA[:, c, :, :], s_rep[:])
            etmp = cr.tile([L_, B, L_], F32, name="cetmp")
            nc.scalar.activation(etmp[:], tmp[:], AF.Exp)
            sex = cr.tile([L_, B], F32, name="sex")
            nc.vector.tensor_reduce(sex[:], etmp[:], AX.X, OP.add)
            nc.scalar.activation(s_fm[:], sex[:], AF.Ln)
            if c < CC - 1:
                replicate(s_fm)

        # logZ_b = LSE_k(s[k,b] + end[k]) + lenoff[b]; out slot0 = sum_b
        send = cr.tile([L_, B], F32, name="send")
        nc.vector.tensor_add(send[:], s_fm[:],
                             end_sb[:, :].to_broadcast([L_, B]))
        eend = cr.tile([L_, B], F32, name="eend")
        nc.scalar.activation(eend[:], send[:], AF.Exp)
        pz = cps1.tile([1, B], F32, name="pz")
        nc.tensor.matmul(pz[:], one17[:], eend[:], start=True, stop=True)
        lz = cr.tile([1, B], F32, name="lz")
        nc.scalar.activation(lz[:], pz[:], AF.Ln)
        nc.vector.tensor_add(lz[:], lz[:], lenoff_sb[:])
        nc.vector.tensor_reduce(out_sb[:, 0:1], lz[:], AX.X, OP.add)

        # ---- numerator parts ----
        def dot_to_slot(vec_lp, slot):
            # vec_lp: [L, 1] -> sum over partitions into out_sb[0, slot]
            pd = cps1.tile([1, 1], F32, name="pd")
            nc.tensor.matmul(pd[:], one17[:], vec_lp[:], start=True,
                             stop=True)
            nc.vector.tensor_copy(out_sb[:, slot:slot + 1], pd[:])

        # e_tag part
        wm_sb = cr.tile([L_, NT], F32, name="wm")
        nc.sync.dma_start(wm_sb[:], d["wmask_in"][:])
        prod = cr.tile([L_, NT], F32, name="prod")
        nc.vector.tensor_mul(prod[:], em[:], wm_sb[:])
        r1 = cr.tile([L_, 1], F32, name="r1")
        nc.vector.tensor_reduce(r1[:], prod[:], AX.X, OP.add)
        dot_to_slot(r1, 1)

        # trans part: C = ohprevM.T-ish contraction, then <C, trans>
        ohp = cr.tile([128, NC2, L_], F32, name="ohp")
        nc.sync.dma_start(ohp[:],
                          d["ohprev_in"][:].rearrange("(c p) l -> p c l",
                                                      p=128))
        ohn = cr.tile([128, NC2, L_], F32, name="ohn")
        nc.sync.dma_start(ohn[:],
                          d["ohnext_in"][:].rearrange("(c p) l -> p c l",
                                                      p=128))
        pC = cps1.tile([L_, L_], F32, name="pC")
        for c2 in range(NC2):
            nc.tensor.matmul(pC[:], ohp[:, c2, :], ohn[:, c2, :],
                             start=(c2 == 0), stop=(c2 == NC2 - 1))
        tC = cr.tile([L_, L_], F32, name="tC")
        nc.vector.tensor_mul(tC[:], pC[:], trans_sb[:])
        r2 = cr.tile([L_, 1], F32, name="r2")
        nc.vector.tensor_reduce(r2[:], tC[:], AX.X, OP.add)
        dot_to_slot(r2, 2)

        # start / end parts
        oh0_sb = cr.tile([L_, B], F32, name="oh0")
        nc.sync.dma_start(oh0_sb[:], d["oh0_in"][:])
        t0 = cr.tile([L_, B], F32, name="t0")
        nc.vector.tensor_mul(t0[:], oh0_sb[:],
                             start_sb[:, :].to_broadcast([L_, B]))
        r3 = cr.tile([L_, 1], F32, name="r3")
        nc.vector.tensor_reduce(r3[:], t0[:], AX.X, OP.add)
        dot_to_slot(r3, 3)

        ohl_sb = cr.tile([L_, B], F32, name="ohl")
        nc.sync.dma_start(ohl_sb[:], d["ohlast_in"][:])
        t4 = cr.tile([L_, B], F32, name="t4")
        nc.vector.tensor_mul(t4[:], ohl_sb[:],
                             end_sb[:, :].to_broadcast([L_, B]))
        r4 = cr.tile([L_, 1], F32, name="r4")
        nc.vector.tensor_reduce(r4[:], t4[:], AX.X, OP.add)
        dot_to_slot(r4, 4)

        nc.sync.dma_start(d["out_d"][:], out_sb[:])


def _prep_core(core, sentences, mask, labels, T):
    """Per-core numpy input prep (index/layout only)."""
    NT = T * B
    NG = NT // 128
    bs = slice(B * core, B * (core + 1))
    sent = np.asarray(sentences[bs], dtype=np.int64)
    msk = np.asarray(mask[bs], dtype=bool)
    lab = np.asarray(labels[bs], dtype=np.int64)
    lens = msk.sum(axis=1).astype(np.int64)

    cols = np.arange(NT)
    tt, bb = cols // B, cols % B
    gidx = sent[bb, tt].astype(np.int32).reshape(NG, 128).T.copy()

    maskf = msk.astype(np.float32)
    lab_t = lab[bb, tt]                      # [NT]
    wmask = np.zeros((L, NT), np.float32)
    wmask[lab_t, cols] = maskf[bb, tt]

    ohprev = np.zeros((NT, L), np.float32)
    ohnext = np.zeros((NT, L), np.float32)
    valid_prev = tt >= 1
    lab_prev = lab[bb[valid_prev], tt[valid_prev] - 1]
    ohprev[cols[valid_prev], lab_prev] = maskf[bb[valid_prev],
                                               tt[valid_prev]]
    ohnext[cols, lab_t] = 1.0

    oh0 = np.zeros((L, B), np.float32)
    oh0[lab[:, 0], np.arange(B)] = 1.0
    ohlast = np.zeros((L, B), np.float32)
    ohlast[lab[np.arange(B), lens - 1], np.arange(B)] = 1.0

    lenoff = ((lens - 1).astype(np.float32) * CREB)[None, :]

    inactive = (tt == 0) | (tt >= lens[bb])
    moff = np.broadcast_to(inactive[None, :], (L, NT)).astype(np.uint8).copy()

    return {"gidx": gidx, "wmask": wmask, "ohprev": ohprev,
            "ohnext": ohnext, "oh0": oh0, "ohlast": ohlast,
            "lenoff": lenoff.astype(np.float32), "moff": moff}


def _prep_shared(emb, lstm_params, W_out, b_out, start_t, end_t, trans):
    def f32c(x):
        return np.ascontiguousarray(np.asarray(x), dtype=np.float32)

    (w1f, wh1f, bf1, w1b, wh1b, bb1), (w2f, wh2f, bf2, w2b, wh2b, bb2) = \
        lstm_params

    def bias_fm(b_):
        return np.ascontiguousarray(f32c(b_).reshape(16, 128).T)

    return {
        "emb": f32c(emb),
        "wih1f": np.ascontiguousarray(f32c(w1f).T),
        "wih1b": np.ascontiguousarray(f32c(w1b).T),
        "whh1f": np.ascontiguousarray(f32c(wh1f).T),
        "whh1b": np.ascontiguousarray(f32c(wh1b).T),
        "wih2f": np.ascontiguousarray(f32c(w2f).T),
        "wih2b": np.ascontiguousarray(f32c(w2b).T),
        "whh2f": np.ascontiguousarray(f32c(wh2f).T),
        "whh2b": np.ascontiguousarray(f32c(wh2b).T),
        "b1f": bias_fm(bf1), "b1b": bias_fm(bb1),
        "b2f": bias_fm(bf2), "b2b": bias_fm(bb2),
        "wout": np.ascontiguousarray(f32c(W_out).T),
        "bout": f32c(b_out).reshape(L, 1),
        "trans": f32c(trans),
        "start": f32c(start_t).reshape(L, 1),
        "end": f32c(end_t).reshape(L, 1),
    }


def run(sentences, mask, labels, emb, lstm_params, W_out, b_out, start_t,
        end_t, trans, T=None, V=None):
    T = T if T is not None else np.asarray(sentences).shape[1]
    V = V if V is not None else np.asarray(emb).shape[0]
    import os
    key = (T, V, os.environ.get("BASS_PHASES", "6"))
    if key not in _CACHE:
        _CACHE[key] = _build(T, V)
    nc = _CACHE[key]

    shared = _prep_shared(emb, lstm_params, W_out, b_out, start_t, end_t,
                          trans)
    in_maps = []
    for core in range(8):
        m = dict(shared)
        m.update(_prep_core(core, sentences, mask, labels, T))
        in_maps.append(m)
    res = run_bass_kernel_spmd(nc, in_maps, list(range(8)))
    total = 0.0
    for core in range(8):
        o = res.results[core]["out"][0]
        total += float(o[0]) - float(o[1] + o[2] + o[3] + o[4])
    return np.float32(total / 64.0)


def kernel(sentences, mask, labels, emb, lstm_params, W_out, b_out, start_t,
           end_t, trans):
    return run(sentences, mask, labels, emb, lstm_params, W_out, b_out,
               start_t, end_t, trans)
